# revision 51
# baseline (speedup 1.0000x reference)
"""Trainium2 Bass kernel for nn_LlamaForSequenceRegression_14336600834254.

2-layer Llama (D=2048, H=16, HD=128, F=5632, LoRA r=16 on q/v) + regression
head, B=2, S=1024, fp32 reference.

Distribution (8 NeuronCores): DP2 x TP4.
  - cores 0-3 process batch 0, cores 4-7 batch 1 (data parallel).
  - within each group of 4: Megatron tensor parallel - Wq/Wk/Wv column
    shards (4 heads/core), Wo row shards, Wgate/Wup column shards
    (F/4=1408), Wdown row shards. AllReduce (bf16) after attn-out and
    after MLP-down, replica_groups=[[0,1,2,3],[4,5,6,7]].
  - embedding gather, the layer-0 input rmsnorm, the layer-0 LoRA
    down-projections, and norm-weight folding are done host-side; all
    device matmuls run in bf16 with fp32 PSUM accumulation; the
    residual stream / softmax / rmsnorm statistics are fp32.

Performance structure (vs the first working version):
  - all weights stored host-side in partition-major layouts so every
    SBUF load is a large contiguous DMA (k-pair / og-pair tiles).
  - layer-0 norm + LoRA down-activations computed on host; hn0 uploads
    gate the first projections, bulk h upload is deferred.
  - bf16 residual stream (validated: adds ~2e-3 rel err).
  - per-half pipeline: q-proj/rope interleaved per head with attention
    so the out-proj AllReduce of half 0 starts as early as possible;
    MLP AllReduces per half overlap the next phase.
  - last-token tail: wq preloaded into SBUF during layer 0; the tail
    q/attention/out-proj/MLP use column-packed PSUM accumulation (one
    has_written group per bank) with one activation op per stage; tail
    MLP weights are preloaded ahead of the AllReduce so their DMAs are
    not FIFO-blocked behind the AR-landing descriptor.
"""

import numpy as np
import ml_dtypes

import concourse.bacc as bacc
import concourse.tile as tile
from concourse import mybir
from concourse import bass_utils

BF16 = ml_dtypes.bfloat16
FP32 = np.float32

V, D, L, H, HD, F, R, ALPHA, B, S, OUT = 32000, 2048, 2, 16, 128, 5632, 16, 32, 2, 1024, 11
EPS = 1e-5
SCALING = ALPHA / R
N_CORES = 8
TP = 4
NH = H // TP          # 4 local heads
DL = NH * HD          # 512 local q/k/v cols
FL = F // TP          # 1408 local mlp cols
KC = D // 128         # 16 contraction chunks
FC = FL // 128        # 11 mlp chunks
TT = 512              # token tile (free dim per matmul)
NT = S // TT          # 2 token tiles
TC = S // 128         # 8 token chunks (128-wide)
OW = TT // TP         # 128 tokens owned per rank per half (seq-parallel)
REPLICA_GROUPS = [[0, 1, 2, 3], [4, 5, 6, 7]]

dt = mybir.dt


def build_program(reps=1, out_groups=4, wo_q="sp", wscale=1.0):
    """Build the SPMD Bass program (identical on all 8 cores; weights differ
    per core via the input maps)."""
    nc = bacc.Bacc(num_devices=N_CORES, debug=False)

    # ---- DRAM I/O ----
    # xT: only the rank-owned token chunks (seq-parallel residual)
    xT = nc.dram_tensor("xT", [128, KC, NT * OW], dt.bfloat16, kind="ExternalInput")
    hsel = nc.dram_tensor("hsel", [128, 1], dt.bfloat16, kind="ExternalInput")
    hn0 = nc.dram_tensor("hn0", [128, KC, S], dt.bfloat16, kind="ExternalInput")
    aqx = nc.dram_tensor("aqx", [R, S], dt.bfloat16, kind="ExternalInput")
    avx = nc.dram_tensor("avx", [R, S], dt.bfloat16, kind="ExternalInput")
    cosT = nc.dram_tensor("cosT", [128, S], dt.bfloat16, kind="ExternalInput")
    sinT = nc.dram_tensor("sinT", [128, S], dt.bfloat16, kind="ExternalInput")
    mstrip = nc.dram_tensor("mstrip", [128, 896], dt.bfloat16, kind="ExternalInput")
    maskbias = nc.dram_tensor("maskbias", [128, TC], dt.float32, kind="ExternalInput")
    mbtail = nc.dram_tensor("mbtail", [128, NH * TC], dt.float32, kind="ExternalInput")
    wreg = nc.dram_tensor("wreg", [128, KC, OUT], dt.bfloat16, kind="ExternalInput")
    breg = nc.dram_tensor("breg", [OUT, 1], dt.float32, kind="ExternalInput")
    W = {}
    for l in range(L):
        W[f"wq{l}"] = nc.dram_tensor(f"wq{l}", [128, KC, DL], dt.bfloat16, kind="ExternalInput")
        W[f"wk{l}"] = nc.dram_tensor(f"wk{l}", [128, KC, DL], dt.bfloat16, kind="ExternalInput")
        W[f"wv{l}"] = nc.dram_tensor(f"wv{l}", [128, KC, DL], dt.bfloat16, kind="ExternalInput")
        if l > 0:
            W[f"aq{l}"] = nc.dram_tensor(f"aq{l}", [KC, 128, R], dt.bfloat16, kind="ExternalInput")
            W[f"av{l}"] = nc.dram_tensor(f"av{l}", [KC, 128, R], dt.bfloat16, kind="ExternalInput")
        W[f"bq{l}"] = nc.dram_tensor(f"bq{l}", [R, DL], dt.bfloat16, kind="ExternalInput")
        W[f"bv{l}"] = nc.dram_tensor(f"bv{l}", [R, DL], dt.bfloat16, kind="ExternalInput")
        W[f"wo{l}"] = nc.dram_tensor(f"wo{l}", [NH, 128, D], dt.bfloat16, kind="ExternalInput")
        # [FC, 128(d-part), KC, 128(f-col)] -> contiguous per-fc SBUF loads
        W[f"wg{l}"] = nc.dram_tensor(f"wg{l}", [FC, 128, KC, 128], dt.bfloat16, kind="ExternalInput")
        W[f"wu{l}"] = nc.dram_tensor(f"wu{l}", [FC, 128, KC, 128], dt.bfloat16, kind="ExternalInput")
        W[f"wd{l}"] = nc.dram_tensor(f"wd{l}", [FC, 128, D], dt.bfloat16, kind="ExternalInput")
    out_dram = nc.dram_tensor("out", [OUT, 1], dt.float32, kind="ExternalOutput")

    with tile.TileContext(nc) as tc:
        with (
            tc.tile_pool(name="persist", bufs=1) as pp,
            tc.tile_pool(name="wts", bufs=4) as wp,
            tc.tile_pool(name="colw", bufs=5) as cwp,
            tc.tile_pool(name="tmp", bufs=3) as tp_,
            tc.tile_pool(name="stage", bufs=2) as stp,
            tc.tile_pool(name="psum", bufs=8, space="PSUM") as ps,
            tc.tile_pool(name="dram", bufs=1, space="DRAM") as dram,
        ):
            f32, bf = dt.float32, dt.bfloat16
            # ---- persistent tiles ----
            # hres: the rank's OWNED residual tokens only ([half t, slot w]
            # = global token t*TT + r*OW + w for TP rank r).
            hres = pp.tile([128, KC, NT * OW], bf, tag="hres")
            hsel_sb = pp.tile([128, 1], bf, tag="hsel")
            hn = pp.tile([128, KC, S], bf, tag="hn")
            cos_sb = pp.tile([128, S], bf, tag="cos")
            sin_sb = pp.tile([128, S], bf, tag="sin")
            mstrip_sb = pp.tile([128, 896], bf, tag="mstrip")
            mb_sb = pp.tile([128, TC], f32, tag="mb")
            mbt_sb = pp.tile([128, NH * TC], f32, tag="mbt")
            oneD_sb = pp.tile([128, 1], bf, tag="oneD")
            ones_bf = pp.tile([128, 1], bf, tag="onesbf")
            eps_sb = pp.tile([1, 1], f32, tag="eps")
            ones_row = pp.tile([1, 128], f32, tag="ones_row")
            qT = pp.tile([128, NH, S], bf, tag="qT")
            kT = pp.tile([128, NH, S], bf, tag="kT")
            vN = pp.tile([128, TC, DL], bf, tag="vN")
            ctxT = pp.tile([128, NH, S], bf, tag="ctxT")
            mT = pp.tile([128, FC, S], bf, tag="mT")
            aqw = pp.tile([128, KC, R], bf, tag="aqw")
            avw = pp.tile([128, KC, R], bf, tag="avw")
            bq_sb = pp.tile([R, DL], bf, tag="bq")
            bv_sb = pp.tile([R, DL], bf, tag="bv")
            aq_sb = pp.tile([R, S], bf, tag="aq")
            av_sb = pp.tile([R, S], bf, tag="av")
            wreg_sb = pp.tile([128, KC, OUT], bf, tag="wreg")
            breg_sb = pp.tile([OUT, 1], f32, tag="breg")
            wq1_sb = pp.tile([128, KC, DL], bf, tag="wq1")

            for _rep in range(reps):
                # ---- constants in ----
                # hn0 first: it gates the first projections
                for k0, k1 in ((0, 2), (2, 4), (4, 8), (8, 12), (12, 16)):
                    nc.sync.dma_start(hn[:, k0:k1, :], hn0[:, k0:k1, :])
                nc.sync.dma_start(cos_sb[:], cosT[:])
                nc.sync.dma_start(sin_sb[:], sinT[:])
                nc.vector.memset(oneD_sb[:], 1.0 / D)
                nc.vector.memset(ones_bf[:], 1.0)
                nc.vector.memset(eps_sb[:], EPS)
                nc.vector.memset(ones_row[:], 1.0)
                nc.sync.dma_start(aq_sb[:], aqx[:])
                nc.sync.dma_start(av_sb[:], avx[:])
                nc.sync.dma_start(hsel_sb[:], hsel[:])

                # DRAM bounce buffers for collectives.  Boundary = partial
                # sums staged rank-major [TP, 128, KC, OW] per half ->
                # ReduceScatter (rank gets its OW-token chunk) -> local
                # add+rmsnorm -> AllGather normed full half back into hn.
                # (HW collectives require contiguous APs, so the staging DMA
                # writes the rank-major layout directly.)
                ar1h_in = [dram.tile([TP, 128, KC, OW], bf, name=f"ar1hi_{t}") for t in range(NT)]
                ar2h_in = [dram.tile([TP, 128, KC, OW], bf, name=f"ar2hi_{t}") for t in range(NT)]
                rs_out = {(b, t): dram.tile([128, KC, OW], bf, name=f"rso_{b}_{t}")
                          for b in (1, 2) for t in range(NT)}
                ag_in = {(b, t): dram.tile([128, KC, OW], bf, name=f"agi_{b}_{t}")
                         for b in (1, 2) for t in range(NT)}
                ag_out = {(b, t): dram.tile([TP, 128, KC, OW], bf, name=f"ago_{b}_{t}")
                          for b in (1, 2) for t in range(NT)}

                def stage_rank_major(dst, k, pso):
                    """PSUM [128, TT] f32 partial -> bf16 stage tile (DVE
                    copy: the Act queue is contended by softmax exps) ->
                    dst[TP, 128, k, OW] rank-major for ReduceScatter."""
                    st = stp.tile([128, TT], bf, tag="stage")
                    nc.vector.tensor_mul(st[:], pso[:],
                                         ones_bf[:].broadcast_to([128, TT]))
                    # rearrange the DRAM side only: SBUF APs must keep the
                    # partition dim leading
                    nc.sync.dma_start(
                        dst[:, :, k, :].transpose([1, 0, 2]),
                        st[:].rearrange("p (r w) -> p r w", r=TP))

                def rs_half(b, t):
                    src = (ar1h_in if b == 1 else ar2h_in)[t]
                    nc.gpsimd.collective_compute(
                        "ReduceScatter", mybir.AluOpType.add,
                        replica_groups=REPLICA_GROUPS,
                        ins=[src.opt()], outs=[rs_out[b, t].opt()],
                    )

                def ag_half(b, t):
                    nc.gpsimd.collective_compute(
                        "AllGather", mybir.AluOpType.bypass,
                        replica_groups=REPLICA_GROUPS,
                        ins=[ag_in[b, t].opt()], outs=[ag_out[b, t].opt()],
                    )

                def boundary_stage(b, t):
                    """Land the rank's summed OW-token chunk, add into hres,
                    rmsnorm it, stage the normed chunk for AllGather.
                    Elementwise work runs on gpsimd (Pool) — the engine is
                    otherwise idle and its queue is where the collectives
                    live, so the ops slot naturally between RS and AG."""
                    cs = slice(t * OW, (t + 1) * OW)
                    lb = tp_.tile([128, KC, OW], bf, tag="rsland", bufs=2,
                                  name=f"rsl_{b}_{t}")
                    nc.sync.dma_start(lb[:], rs_out[b, t][:])
                    nc.gpsimd.tensor_add(hres[:, :, cs], hres[:, :, cs], lb[:])
                    sumsq = ps.tile([128, OW], f32, tag="psum", name=f"nsum_{b}_{t}")
                    for k in range(KC):
                        sq = tp_.tile([128, OW], bf, tag="sqc", bufs=2,
                                      name=f"nsq_{b}_{t}_{k}")
                        nc.gpsimd.tensor_mul(sq[:], hres[:, k, cs], hres[:, k, cs])
                        nc.tensor.matmul(sumsq[0:1, :], oneD_sb[:], sq[:],
                                         start=(k == 0), stop=(k == KC - 1))
                    rsl = tp_.tile([1, OW], f32, tag="vec1c", bufs=2,
                                   name=f"rslv_{b}_{t}")
                    nc.scalar.activation(rsl[0:1, :], sumsq[0:1, :],
                                         mybir.ActivationFunctionType.Sqrt, bias=eps_sb[:])
                    nc.vector.reciprocal(rsl[0:1, :], rsl[0:1, :])
                    psrs = ps.tile([128, OW], f32, tag="psum", name=f"psrs_{b}_{t}")
                    nc.tensor.matmul(psrs[:], ones_row[:], rsl[0:1, :],
                                     start=True, stop=True)
                    # gpsimd cannot read PSUM - bounce through SBUF
                    rs_sb = tp_.tile([128, OW], f32, tag="rs_sb", bufs=2,
                                     name=f"rssb_{b}_{t}")
                    nc.scalar.copy(rs_sb[:], psrs[:])
                    hnc = tp_.tile([128, KC, OW], bf, tag="hnc", bufs=2,
                                   name=f"hnc_{b}_{t}")
                    for k in range(KC):
                        nc.gpsimd.tensor_mul(hnc[:, k, :], hres[:, k, cs], rs_sb[:])
                    nc.sync.dma_start(ag_in[b, t][:], hnc[:])

                def land_hn(b, t):
                    for r in range(TP):
                        rs_ = slice(t * TT + r * OW, t * TT + (r + 1) * OW)
                        nc.sync.dma_start(hn[:, :, rs_], ag_out[b, t][r])

                def rope_from_psum(psq, dst, hc, t):
                    """Apply RoPE to psum [128,TT] (one head, token tile t) and
                    write bf16 into dst[:, hc, t*TT:...]."""
                    ts_ = slice(t * TT, (t + 1) * TT)
                    # cos/sin tiles carry the same 64-row table duplicated into
                    # both partition halves so every SB+SB operand pair below is
                    # base-partition aligned.
                    t2 = tp_.tile([128, TT], bf, tag="ropetB", bufs=2)
                    t4 = tp_.tile([128, TT], bf, tag="ropetB", bufs=2)
                    nc.vector.tensor_mul(dst[0:64, hc, ts_], psq[0:64, :], cos_sb[0:64, ts_])
                    nc.vector.tensor_mul(t2[0:64, :], psq[64:128, :], sin_sb[0:64, ts_])
                    nc.vector.tensor_sub(dst[0:64, hc, ts_], dst[0:64, hc, ts_], t2[0:64, :])
                    nc.vector.tensor_mul(dst[64:128, hc, ts_], psq[64:128, :], cos_sb[64:128, ts_])
                    nc.vector.tensor_mul(t4[64:128, :], psq[0:64, :], sin_sb[64:128, ts_])
                    nc.vector.tensor_add(dst[64:128, hc, ts_], dst[64:128, hc, ts_], t4[64:128, :])

                def qk_attn_out_half(l, t):
                    """q-proj for half t, then attention per head with score
                    matmuls batched in groups (separate PSUM banks) so the
                    PE->exp->accumulate chain pipelines instead of
                    serializing per key-chunk."""
                    ts_ = slice(t * TT, (t + 1) * TT)
                    jmax = (t + 1) * (TT // 128)
                    psq = [ps.tile([128, TT], f32, tag="psum", name=f"psqf_{hc}_{t}")
                           for hc in range(NH)]
                    for k2 in range(KC // 2):
                        wt = wp.tile([128, 2, DL], bf, tag="wqkv", bufs=3,
                                     name=f"wqf_{k2}_{t}")
                        nc.sync.dma_start(wt[:], W[f"wq{l}"][:, 2 * k2:2 * k2 + 2, :])
                        for kk in range(2):
                            k = 2 * k2 + kk
                            for hc in range(NH):
                                nc.tensor.matmul(
                                    psq[hc][:], wt[:, kk, hc * HD:(hc + 1) * HD],
                                    hn[:, k, ts_], start=(k == 0), stop=False,
                                )
                    # rope all heads first: frees the psq banks before the
                    # score groups need them
                    for hh in range(NH):
                        nc.tensor.matmul(
                            psq[hh][:], bq_sb[:, hh * HD:(hh + 1) * HD],
                            aq_sb[:, ts_], start=False, stop=True,
                        )
                        rope_from_psum(psq[hh], qT, hh, t)
                    G = 2
                    for hh in range(NH):
                        psd = ps.tile([128, TT], f32, tag="psum")
                        psc = ps.tile([128, TT], f32, tag="psum")
                        groups = [list(range(j0, min(j0 + G, jmax)))
                                  for j0 in range(0, jmax, G)]
                        sc, exs = {}, {}

                        def scores(g):
                            for j in groups[g]:
                                pss = ps.tile([128, TT], f32, tag="psum",
                                              name=f"pss_{t}_{hh}_{j}")
                                nc.tensor.matmul(
                                    pss[:], kT[:, hh, j * 128:(j + 1) * 128],
                                    qT[:, hh, ts_], start=True, stop=True,
                                )
                                sc[j] = pss

                        def exps(g):
                            for j in groups[g]:
                                ex = tp_.tile([128, TT], bf, tag="exj", bufs=4,
                                              name=f"ex_{t}_{hh}_{j}")
                                nc.scalar.activation(
                                    ex[:], sc[j][:],
                                    mybir.ActivationFunctionType.Exp,
                                    bias=mb_sb[:, j:j + 1], scale=1.0,
                                )
                                off = t * TT - j * 128
                                if off < 128:
                                    # diagonal tile: multiply 0/1 causal strip
                                    # allowed iff p <= f + off
                                    nc.vector.tensor_mul(
                                        ex[:], ex[:],
                                        mstrip_sb[:, 384 + off:896 + off],
                                    )
                                exs[j] = ex

                        def accums(g):
                            for j in groups[g]:
                                nc.tensor.matmul(
                                    psd[0:1, :], ones_bf[:], exs[j][:],
                                    start=(j == 0), stop=(j == jmax - 1),
                                )
                                nc.tensor.matmul(
                                    psc[:], vN[:, j, hh * HD:(hh + 1) * HD],
                                    exs[j][:],
                                    start=(j == 0), stop=(j == jmax - 1),
                                )

                        scores(0)
                        for g in range(len(groups)):
                            exps(g)
                            if g + 1 < len(groups):
                                scores(g + 1)
                            accums(g)
                        rdl = tp_.tile([1, TT], f32, tag="vec1", bufs=2,
                                       name=f"rdl_{t}_{hh}")
                        nc.vector.reciprocal(rdl[0:1, :], psd[0:1, :])
                        # broadcast 1/denom over partitions via PE outer
                        # product: keeps the Pool queue free for collectives
                        psrd = ps.tile([128, TT], f32, tag="psum",
                                       name=f"psrd_{t}_{hh}")
                        nc.tensor.matmul(psrd[:], ones_row[:], rdl[0:1, :],
                                         start=True, stop=True)
                        # HW: only one PSUM operand per vector op - bounce
                        # the broadcast through SBUF
                        rden_sb = tp_.tile([128, TT], f32, tag="rden", bufs=2,
                                           name=f"rden_{t}_{hh}")
                        nc.scalar.copy(rden_sb[:], psrd[:])
                        nc.vector.tensor_mul(ctxT[:, hh, ts_], psc[:], rden_sb[:])
                    out_proj_half(l, t)

                def out_proj_half(l, t):
                    """attn partial for token half t -> ar1h_in[t]."""
                    go = out_groups
                    wo_eng = {"sp": nc.sync, "act": nc.scalar,
                              "pool": nc.gpsimd}[wo_q]
                    for og in range(16 // go):
                        pso = [ps.tile([128, TT], f32, tag="psum",
                                       name=f"pso_{og}_{oi}_{t}")
                               for oi in range(go)]
                        for hc in range(NH):
                            wt = wp.tile([128, go * 128], bf, tag="wo", bufs=4,
                                         name=f"wo_t_{og}_{hc}_{t}")
                            wo_eng.dma_start(
                                wt[:], W[f"wo{l}"][hc][:, og * go * 128:(og + 1) * go * 128])
                            for oi in range(go):
                                nc.tensor.matmul(
                                    pso[oi][:],
                                    wt[:, oi * 128:(oi + 1) * 128],
                                    ctxT[:, hc, t * TT:(t + 1) * TT],
                                    start=(hc == 0), stop=(hc == NH - 1),
                                )
                        for oi in range(go):
                            stage_rank_major(ar1h_in[t], og * go + oi, pso[oi])

                def lora_down_half(aw, dst, t):
                    psa = ps.tile([128, TT], f32, tag="psum", name=f"psld_{t}")
                    for k in range(KC):
                        nc.tensor.matmul(
                            psa[0:R, :], aw[:, k, :], hn[:, k, t * TT:(t + 1) * TT],
                            start=(k == 0), stop=(k == KC - 1),
                        )
                    nc.scalar.copy(dst[:, t * TT:(t + 1) * TT], psa[0:R, :])

                def k_proj_half(wname, dst, t):
                    psq = [ps.tile([128, TT], f32, tag="psum", name=f"psqh_{hc}_{t}")
                           for hc in range(NH)]
                    for k2 in range(KC // 2):
                        wt = wp.tile([128, 2, DL], bf, tag="wqkv", bufs=3,
                                     name=f"wkh_{k2}_{t}")
                        nc.sync.dma_start(wt[:], W[wname][:, 2 * k2:2 * k2 + 2, :])
                        for kk in range(2):
                            k = 2 * k2 + kk
                            for hc in range(NH):
                                nc.tensor.matmul(
                                    psq[hc][:], wt[:, kk, hc * HD:(hc + 1) * HD],
                                    hn[:, k, t * TT:(t + 1) * TT],
                                    start=(k == 0), stop=(k == KC - 1),
                                )
                    for hc in range(NH):
                        rope_from_psum(psq[hc], dst, hc, t)

                def v_proj_half(l, t):
                    psv = [ps.tile([128, DL], f32, tag="psum", name=f"psvh_{c}_{t}")
                           for c in range(4)]
                    for k2 in range(KC // 2):
                        wt = wp.tile([128, 2, DL], bf, tag="wqkv", bufs=3,
                                     name=f"wvh_{k2}_{t}")
                        nc.sync.dma_start(wt[:], W[f"wv{l}"][:, 2 * k2:2 * k2 + 2, :])
                        for kk in range(2):
                            k = 2 * k2 + kk
                            for ci in range(4):
                                c = t * 4 + ci
                                nc.tensor.matmul(
                                    psv[ci][:], hn[:, k, c * 128:(c + 1) * 128],
                                    wt[:, kk, :],
                                    start=(k == 0), stop=False,
                                )
                    for ci in range(4):
                        c = t * 4 + ci
                        nc.tensor.matmul(
                            psv[ci][:], av_sb[:, c * 128:(c + 1) * 128], bv_sb[:],
                            start=False, stop=True,
                        )
                        nc.scalar.copy(vN[:, c, :], psv[ci][:])

                def mlp_gate_up_half(l, t, fcs=None):
                    ts_ = slice(t * TT, (t + 1) * TT)
                    for fc in (range(FC) if fcs is None else fcs):
                        wg_h = [cwp.tile([128, KC // 2, 128], bf, tag="wgcol",
                                         name=f"wg_{fc}_{t}_{hh}") for hh in range(2)]
                        wu_h = [cwp.tile([128, KC // 2, 128], bf, tag="wucol",
                                         name=f"wu_{fc}_{t}_{hh}") for hh in range(2)]
                        for hh in range(2):
                            ksl = slice(hh * (KC // 2), (hh + 1) * (KC // 2))
                            nc.sync.dma_start(wg_h[hh][:], W[f"wg{l}"][fc][:, ksl, :])
                            nc.sync.dma_start(wu_h[hh][:], W[f"wu{l}"][fc][:, ksl, :])
                        psg = ps.tile([128, TT], f32, tag="psum", name=f"psg_{fc}_{t}")
                        psu = ps.tile([128, TT], f32, tag="psum", name=f"psu_{fc}_{t}")
                        for k in range(KC):
                            nc.tensor.matmul(psg[:], wg_h[k // 8][:, k % 8, :],
                                             hn[:, k, ts_],
                                             start=(k == 0), stop=(k == KC - 1))
                            nc.tensor.matmul(psu[:], wu_h[k // 8][:, k % 8, :],
                                             hn[:, k, ts_],
                                             start=(k == 0), stop=(k == KC - 1))
                        sg = tp_.tile([128, TT], bf, tag="silu", bufs=2, name=f"sg_{fc}_{t}")
                        nc.scalar.activation(sg[:], psg[:], mybir.ActivationFunctionType.Silu)
                        nc.vector.tensor_mul(mT[:, fc, ts_], sg[:], psu[:])

                def mlp_down_half(l, t):
                    ts_ = slice(t * TT, (t + 1) * TT)
                    for og in range(4):
                        pso = [ps.tile([128, TT], f32, tag="psum",
                                       name=f"psd_{og}_{oi}_{t}")
                               for oi in range(4)]
                        for kc in range(FC):
                            wt = wp.tile([128, TT], bf, tag="wd", bufs=4,
                                         name=f"wd_t_{og}_{kc}_{t}")
                            nc.sync.dma_start(
                                wt[:], W[f"wd{l}"][kc][:, og * 512:(og + 1) * 512])
                            for oi in range(4):
                                nc.tensor.matmul(
                                    pso[oi][:],
                                    wt[:, oi * 128:(oi + 1) * 128],
                                    mT[:, kc, ts_],
                                    start=(kc == 0), stop=(kc == FC - 1),
                                )
                        for oi in range(4):
                            stage_rank_major(ar2h_in[t], og * 4 + oi, pso[oi])

                # ---------- last-token ("tail") variants for the final layer ----
                # The residual at token S-1 lives only on TP-rank 3 (its last
                # owned slot).  hsel (1.0 on rank 3, else 0) folds it into the
                # first tail AllReduce so every rank gets h_last + attn_sum.
                ar_in_s = dram.tile([128, KC, 1], bf)
                ar_out_s = dram.tile([128, KC, 1], bf)
                ar_in_s2 = dram.tile([128, KC, 1], bf)
                ar_out_s2 = dram.tile([128, KC, 1], bf)
                q_last = pp.tile([128, NH], bf, tag="q_last")
                ctx_last = pp.tile([128, NH], bf, tag="ctx_last")
                hn_last = pp.tile([128, KC, 1], bf, tag="hn_last")
                h_last1 = pp.tile([128, KC, 1], bf, tag="h_last1")
                m_last = pp.tile([128, FC], bf, tag="m_last")
                exps_tail = pp.tile([128, NH * TC], bf, tag="exps_tail")

                def q_proj_tail(l):
                    psq = ps.tile([128, TT], f32, tag="psum", name="psqt")
                    for k in range(KC):
                        for hc in range(NH):
                            # one has_written group for the whole bank: start
                            # clears the full bank, so only the very first
                            # matmul may set it (per-element overwrite covers
                            # each column's first write)
                            nc.tensor.matmul(
                                psq[:, hc:hc + 1],
                                wq1_sb[:, k, hc * HD:(hc + 1) * HD],
                                hn[:, k, S - 1:S],
                                start=(k == 0 and hc == 0), stop=False,
                            )
                    for hc in range(NH):
                        nc.tensor.matmul(
                            psq[:, hc:hc + 1], bq_sb[:, hc * HD:(hc + 1) * HD],
                            aq_sb[:, S - 1:S], start=False, stop=(hc == NH - 1),
                        )
                    # batched rope over all NH columns
                    t2 = tp_.tile([128, NH], f32, tag="ropetS", bufs=2)
                    t4 = tp_.tile([128, NH], f32, tag="ropetS", bufs=2)
                    cb_lo = cos_sb[0:64, S - 1:S].broadcast_to([64, NH])
                    cb_hi = cos_sb[64:128, S - 1:S].broadcast_to([64, NH])
                    sb_lo = sin_sb[0:64, S - 1:S].broadcast_to([64, NH])
                    sb_hi = sin_sb[64:128, S - 1:S].broadcast_to([64, NH])
                    nc.vector.tensor_mul(q_last[0:64, :], psq[0:64, 0:NH], cb_lo)
                    nc.vector.tensor_mul(t2[0:64, :], psq[64:128, 0:NH], sb_lo)
                    nc.vector.tensor_sub(q_last[0:64, :], q_last[0:64, :], t2[0:64, :])
                    nc.vector.tensor_mul(q_last[64:128, :], psq[64:128, 0:NH], cb_hi)
                    nc.vector.tensor_mul(t4[64:128, :], psq[0:64, 0:NH], sb_hi)
                    nc.vector.tensor_add(q_last[64:128, :], q_last[64:128, :], t4[64:128, :])

                def attention_tail():
                    # last token attends to every key: no causal strip needed.
                    # scores for all heads land as columns [keys-part, hh*TC+j].
                    pss = ps.tile([128, TT], f32, tag="psum", name="psst")
                    for hh in range(NH):
                        for j in range(TC):
                            nc.tensor.matmul(
                                pss[:, hh * TC + j:hh * TC + j + 1],
                                kT[:, hh, j * 128:(j + 1) * 128],
                                q_last[:, hh:hh + 1],
                                start=(hh == 0 and j == 0),
                                stop=(hh == NH - 1 and j == TC - 1),
                            )
                    # attention-mask bias per key (zeros for an all-ones mask)
                    nc.vector.tensor_add(pss[:, 0:NH * TC], pss[:, 0:NH * TC],
                                         mbt_sb[:])
                    nc.scalar.activation(exps_tail[:], pss[:, 0:NH * TC],
                                         mybir.ActivationFunctionType.Exp)
                    psd = ps.tile([128, TT], f32, tag="psum", name="psdt")
                    psc = ps.tile([128, TT], f32, tag="psum", name="psct")
                    for hh in range(NH):
                        for j in range(TC):
                            first = hh == 0 and j == 0
                            last = hh == NH - 1 and j == TC - 1
                            nc.tensor.matmul(psd[0:1, hh:hh + 1], ones_bf[:],
                                             exps_tail[:, hh * TC + j:hh * TC + j + 1],
                                             start=first, stop=last)
                            nc.tensor.matmul(psc[:, hh:hh + 1],
                                             vN[:, j, hh * HD:(hh + 1) * HD],
                                             exps_tail[:, hh * TC + j:hh * TC + j + 1],
                                             start=first, stop=last)
                    rd = tp_.tile([1, NH], f32, tag="rd_tail", bufs=2)
                    nc.vector.reciprocal(rd[:], psd[0:1, 0:NH])
                    rdb = tp_.tile([128, NH], f32, tag="rdb_tail", bufs=2)
                    nc.gpsimd.partition_broadcast(rdb[:], rd[:])
                    nc.vector.tensor_mul(ctx_last[:], psc[:, 0:NH], rdb[:])

                def out_proj_tail(l):
                    pso = ps.tile([128, KC], f32, tag="psum", name="psot")
                    for hc in range(NH):
                        wt = wp.tile([128, D], bf, tag="wdtail", bufs=3,
                                     name=f"wot_{hc}")
                        nc.sync.dma_start(wt[:], W[f"wo{l}"][hc])
                        for oc in range(KC):
                            nc.tensor.matmul(
                                pso[:, oc:oc + 1],
                                wt[:, oc * 128:(oc + 1) * 128],
                                ctx_last[:, hc:hc + 1],
                                start=(hc == 0 and oc == 0),
                                stop=(hc == NH - 1 and oc == KC - 1),
                            )
                    st = stp.tile([128, KC], bf, tag="stage_tail", bufs=2)
                    nc.scalar.copy(st[:], pso[:, 0:KC])
                    # + hsel * h_last (residual contributed by rank 3 only)
                    hl = tp_.tile([128, KC], bf, tag="hseltail")
                    nc.vector.tensor_mul(hl[:], hres[:, :, NT * OW - 1],
                                         hsel_sb[:].broadcast_to([128, KC]))
                    nc.vector.tensor_add(st[:], st[:], hl[:])
                    nc.sync.dma_start(ar_in_s[:, :, 0], st[:])

                def ar_tail(ain, aout, dst):
                    nc.gpsimd.collective_compute(
                        "AllReduce", mybir.AluOpType.add,
                        replica_groups=REPLICA_GROUPS,
                        ins=[ain.opt()], outs=[aout.opt()],
                    )
                    nc.sync.dma_start(dst[:], aout[:])

                def norm_tail_to(src, dst):
                    psl = ps.tile([128, TT], f32, tag="psum", name="psl_norm_tail")
                    sqt = tp_.tile([128, KC, 1], bf, tag="sqlast")
                    nc.scalar.activation(sqt[:], src[:],
                                         mybir.ActivationFunctionType.Square)
                    for k in range(KC):
                        nc.tensor.matmul(psl[0:1, 0:1], oneD_sb[:], sqt[:, k, :],
                                         start=(k == 0), stop=(k == KC - 1))
                    rst = tp_.tile([1, 1], f32, tag="rst_tail")
                    nc.scalar.activation(rst[:], psl[0:1, 0:1],
                                         mybir.ActivationFunctionType.Sqrt, bias=eps_sb[:])
                    nc.vector.reciprocal(rst[:], rst[:])
                    rstb = tp_.tile([128, 1], f32, tag="rstb_tail")
                    nc.gpsimd.partition_broadcast(rstb[:], rst[:])
                    nc.vector.tensor_mul(
                        dst[:], src[:],
                        rstb[:, :, None].broadcast_to([128, 1, 1]).broadcast_to([128, KC, 1]),
                    )

                def mlp_tail_preload(l, nfc=4, nkc=2):
                    pre = {}
                    for fc in range(nfc):
                        wg_h = [cwp.tile([128, KC // 2, 128], bf, tag="wgcol",
                                         name=f"wgt_{fc}_{hh}") for hh in range(2)]
                        wu_h = [cwp.tile([128, KC // 2, 128], bf, tag="wucol",
                                         name=f"wut_{fc}_{hh}") for hh in range(2)]
                        for hh in range(2):
                            ksl = slice(hh * (KC // 2), (hh + 1) * (KC // 2))
                            nc.sync.dma_start(wg_h[hh][:], W[f"wg{l}"][fc][:, ksl, :])
                            nc.sync.dma_start(wu_h[hh][:], W[f"wu{l}"][fc][:, ksl, :])
                        pre[fc] = (wg_h, wu_h)
                    pre["wd"] = []
                    for kc in range(nkc):
                        wt = wp.tile([128, D], bf, tag="wdtail", bufs=3,
                                     name=f"wdtp_{kc}")
                        nc.sync.dma_start(wt[:], W[f"wd{l}"][kc])
                        pre["wd"].append(wt)
                    return pre

                def mlp_tail(l, pre):
                    norm_tail_to(h_last1, hn_last)
                    psg = ps.tile([128, TT], f32, tag="psum", name="psgt")
                    psu = ps.tile([128, TT], f32, tag="psum", name="psut")
                    for fc in range(FC):
                        if fc in pre:
                            wg_h, wu_h = pre[fc]
                        else:
                            wg_h = [cwp.tile([128, KC // 2, 128], bf, tag="wgcol",
                                             name=f"wgt_{fc}_{hh}") for hh in range(2)]
                            wu_h = [cwp.tile([128, KC // 2, 128], bf, tag="wucol",
                                             name=f"wut_{fc}_{hh}") for hh in range(2)]
                            for hh in range(2):
                                ksl = slice(hh * (KC // 2), (hh + 1) * (KC // 2))
                                nc.sync.dma_start(wg_h[hh][:], W[f"wg{l}"][fc][:, ksl, :])
                                nc.sync.dma_start(wu_h[hh][:], W[f"wu{l}"][fc][:, ksl, :])
                        for k in range(KC):
                            first = fc == 0 and k == 0
                            last = fc == FC - 1 and k == KC - 1
                            nc.tensor.matmul(psg[:, fc:fc + 1],
                                             wg_h[k // 8][:, k % 8, :],
                                             hn_last[:, k, :],
                                             start=first, stop=last)
                            nc.tensor.matmul(psu[:, fc:fc + 1],
                                             wu_h[k // 8][:, k % 8, :],
                                             hn_last[:, k, :],
                                             start=first, stop=last)
                    sgt = tp_.tile([128, FC], bf, tag="silut", bufs=2)
                    nc.scalar.activation(sgt[:], psg[:, 0:FC],
                                         mybir.ActivationFunctionType.Silu)
                    nc.vector.tensor_mul(m_last[:], sgt[:], psu[:, 0:FC])
                    psdn = ps.tile([128, KC], f32, tag="psum", name="psdt2")
                    for kc in range(FC):
                        if kc < len(pre["wd"]):
                            wt = pre["wd"][kc]
                        else:
                            wt = wp.tile([128, D], bf, tag="wdtail", bufs=3,
                                         name=f"wdt_{kc}")
                            nc.sync.dma_start(wt[:], W[f"wd{l}"][kc])
                        for oc in range(KC):
                            nc.tensor.matmul(
                                psdn[:, oc:oc + 1],
                                wt[:, oc * 128:(oc + 1) * 128],
                                m_last[:, kc:kc + 1],
                                start=(kc == 0 and oc == 0),
                                stop=(kc == FC - 1 and oc == KC - 1),
                            )
                    st = stp.tile([128, KC], bf, tag="stage_tail", bufs=2)
                    nc.scalar.copy(st[:], psdn[:, 0:KC])
                    nc.sync.dma_start(ar_in_s2[:, :, 0], st[:])

                # ================= layer 0 (full sequence) =================
                # per-half pipeline: k/v/q/attn/out of half 0 complete before
                # half 1 starts, so the boundary-1 collectives begin ASAP.
                # tile_wait_until values are scheduling hints only (the
                # scheduler's internal sim underestimates collective latency
                # and would otherwise head-of-line-block engine queues).
                l = 0
                nc.sync.dma_start(bq_sb[:], W[f"bq{l}"][:])
                nc.sync.dma_start(bv_sb[:], W[f"bv{l}"][:])
                nc.sync.dma_start(mstrip_sb[:], mstrip[:])
                nc.sync.dma_start(mb_sb[:], maskbias[:])
                k_proj_half(f"wk{l}", kT, 0)
                v_proj_half(l, 0)
                nc.sync.dma_start(mbt_sb[:], mbtail[:])
                nc.sync.dma_start(breg_sb[:], breg[:])
                nc.sync.dma_start(wreg_sb[:], wreg[:])
                for k4 in range(4):
                    nc.sync.dma_start(hres[:, 4 * k4:4 * (k4 + 1), :],
                                      xT[:, 4 * k4:4 * (k4 + 1), :])
                qk_attn_out_half(l, 0)
                rs_half(1, 0)
                k_proj_half(f"wk{l}", kT, 1)
                v_proj_half(l, 1)
                # boundary-1 half-0 between kv-h1 and attn-h1: its ~1us of PE
                # work slots into the PE queue while attn h1's inputs land
                with tc.tile_wait_until(wscale * 0.175):
                    boundary_stage(1, 0)
                with tc.tile_wait_until(wscale * 0.20):
                    ag_half(1, 0)
                with tc.tile_wait_until(wscale * 0.275):
                    land_hn(1, 0)
                qk_attn_out_half(l, 1)
                with tc.tile_wait_until(wscale * 0.27):
                    rs_half(1, 1)
                nc.sync.dma_start(wq1_sb[:], W["wq1"][:])
                with tc.tile_wait_until(wscale * 0.285):
                    mlp_gate_up_half(l, 0)
                with tc.tile_wait_until(wscale * 0.31):
                    boundary_stage(1, 1)
                with tc.tile_wait_until(wscale * 0.32):
                    ag_half(1, 1)
                with tc.tile_wait_until(wscale * 0.365):
                    mlp_down_half(l, 0)
                with tc.tile_wait_until(wscale * 0.405):
                    rs_half(2, 0)
                with tc.tile_wait_until(wscale * 0.39):
                    land_hn(1, 1)
                with tc.tile_wait_until(wscale * 0.40):
                    mlp_gate_up_half(l, 1, fcs=range(0, 7))
                with tc.tile_wait_until(wscale * 0.445):
                    boundary_stage(2, 0)
                with tc.tile_wait_until(wscale * 0.455):
                    ag_half(2, 0)
                with tc.tile_wait_until(wscale * 0.45):
                    mlp_gate_up_half(l, 1, fcs=range(7, FC))
                    mlp_down_half(l, 1)
                with tc.tile_wait_until(wscale * 0.525):
                    rs_half(2, 1)

                # ================= layer 1 (tail layer) =================
                l = 1
                for k in range(KC):
                    nc.sync.dma_start(aqw[:, k, :], W[f"aq{l}"][k])
                    nc.sync.dma_start(avw[:, k, :], W[f"av{l}"][k])
                nc.sync.dma_start(bq_sb[:], W[f"bq{l}"][:])
                nc.sync.dma_start(bv_sb[:], W[f"bv{l}"][:])
                with tc.tile_wait_until(wscale * 0.525):
                    land_hn(2, 0)
                with tc.tile_wait_until(wscale * 0.53):
                    lora_down_half(avw, av_sb, 0)
                    k_proj_half(f"wk{l}", kT, 0)
                    v_proj_half(l, 0)
                with tc.tile_wait_until(wscale * 0.56):
                    boundary_stage(2, 1)
                with tc.tile_wait_until(wscale * 0.575):
                    ag_half(2, 1)
                with tc.tile_wait_until(wscale * 0.645):
                    land_hn(2, 1)
                with tc.tile_wait_until(wscale * 0.655):
                    lora_down_half(avw, av_sb, 1)
                    lora_down_half(aqw, aq_sb, 1)
                    k_proj_half(f"wk{l}", kT, 1)
                    v_proj_half(l, 1)
                with tc.tile_wait_until(wscale * 0.71):
                    q_proj_tail(l)
                    attention_tail()
                    out_proj_tail(l)
                    pre = mlp_tail_preload(l)
                with tc.tile_wait_until(wscale * 0.73):
                    ar_tail(ar_in_s, ar_out_s, h_last1)
                    mlp_tail(l, pre)
                with tc.tile_wait_until(wscale * 0.79):
                    lb2 = tp_.tile([128, KC, 1], bf, tag="ar_land2")
                    ar_tail(ar_in_s2, ar_out_s2, lb2)

                    # ============= final norm + head (last token only) ========
                    hfin = tp_.tile([128, KC, 1], bf, tag="hfin")
                    nc.vector.tensor_add(hfin[:], h_last1[:], lb2[:])
                    hl = tp_.tile([128, KC, 1], bf, tag="hlast")
                    norm_tail_to(hfin, hl)
                    pso = ps.tile([128, TT], f32, tag="psum")
                    for k in range(KC):
                        nc.tensor.matmul(pso[0:OUT, 0:1], wreg_sb[:, k, :], hl[:, k, :],
                                         start=(k == 0), stop=(k == KC - 1))
                    ot = tp_.tile([OUT, 1], f32, tag="outt")
                    nc.vector.tensor_add(ot[:], pso[0:OUT, 0:1], breg_sb[:])
                    nc.sync.dma_start(out_dram[:], ot[:])

    nc.finalize()
    return nc


_CACHED = {}


def _get_program():
    if "nc" not in _CACHED:
        _CACHED["nc"] = build_program()
    return _CACHED["nc"]


def _host_prepare(inputs):
    """Fold norms/scales into weights, gather embeddings, build the 8
    per-core input maps."""
    ids = np.asarray(inputs["input_ids"]).astype(np.int64)        # [B,S]
    amask = np.asarray(inputs["attention_mask"]).astype(np.int64)  # [B,S]
    embed = np.asarray(inputs["embed"], FP32)

    inv_sqrt_hd = FP32(1.0 / np.sqrt(HD))

    # rope tables (half: both halves identical)
    inv = 1.0 / (10000.0 ** (np.arange(0, HD, 2, dtype=np.float64) / HD))
    ang = (np.arange(S, dtype=np.float64)[:, None] * inv[None, :])  # [S,64]
    cos64 = np.cos(ang).T.astype(BF16)  # [64,S]
    sin64 = np.sin(ang).T.astype(BF16)
    cosT = np.concatenate([cos64, cos64], axis=0).copy()  # [128,S]
    sinT = np.concatenate([sin64, sin64], axis=0).copy()

    # causal strip [128, 896]: strip[p,u] = 1 if (u-384) >= p else 0
    u = np.arange(896)[None, :]
    p = np.arange(128)[:, None]
    mstrip = ((u - 384) >= p).astype(BF16)

    per_core = []
    common = {}

    def fold(l):
        g1 = np.asarray(inputs["norm1"], FP32)[l][:, None]
        g2 = np.asarray(inputs["norm2"], FP32)[l][:, None]
        wq = np.asarray(inputs["Wq"], FP32)[l] * g1 * inv_sqrt_hd
        wk = np.asarray(inputs["Wk"], FP32)[l] * g1
        wv = np.asarray(inputs["Wv"], FP32)[l] * g1
        aq = np.asarray(inputs["Aq"], FP32)[l] * g1
        av = np.asarray(inputs["Av"], FP32)[l] * g1
        bq = np.asarray(inputs["Bq"], FP32)[l] * (SCALING * inv_sqrt_hd)
        bv = np.asarray(inputs["Bv"], FP32)[l] * SCALING
        wo = np.asarray(inputs["Wo"], FP32)[l]
        wg = np.asarray(inputs["Wgate"], FP32)[l] * g2
        wu = np.asarray(inputs["Wup"], FP32)[l] * g2
        wd = np.asarray(inputs["Wdown"], FP32)[l]
        return wq, wk, wv, aq, av, bq, bv, wo, wg, wu, wd

    folded = [fold(l) for l in range(L)]
    wregf = (np.asarray(inputs["Wreg"], FP32) * np.asarray(inputs["norm_f"], FP32)[:, None])
    common["wreg"] = np.ascontiguousarray(
        wregf.reshape(KC, 128, OUT).transpose(1, 0, 2)).astype(BF16)
    common["breg"] = np.asarray(inputs["breg"], FP32).reshape(OUT, 1)
    common["cosT"] = cosT
    common["sinT"] = sinT
    common["mstrip"] = mstrip

    in_maps = []
    for c in range(N_CORES):
        b = c // TP      # batch index (DP group)
        r = c % TP       # TP rank within group
        m = dict(common)
        # embedding gather, transposed: [D,S] -> [16,128,S] -> [128,16,S]
        x_ds = embed[ids[b]].T.astype(BF16).astype(FP32)  # [D, S], bf16-rounded
        xt = x_ds.reshape(KC, 128, S).transpose(1, 0, 2)
        # rank-owned residual chunks only (seq-parallel): half t, tokens
        # [t*TT + r*OW, +OW)
        own = np.concatenate(
            [xt[:, :, t * TT + r * OW:t * TT + (r + 1) * OW] for t in range(NT)],
            axis=2)
        m["xT"] = np.ascontiguousarray(own).astype(BF16)
        m["hsel"] = np.full((128, 1), 1.0 if r == TP - 1 else 0.0, dtype=BF16)
        # layer-0 rmsnorm done on host (g1 folded into weights)
        rs0 = 1.0 / np.sqrt((x_ds ** 2).mean(axis=0) + EPS)
        hn0_ds = (x_ds * rs0[None, :]).astype(BF16)  # [D,S]
        m["hn0"] = np.ascontiguousarray(
            hn0_ds.reshape(KC, 128, S).transpose(1, 0, 2))
        # layer-0 LoRA down-activations on host
        aq_f = folded[0][3]  # aq (g1-folded) [D, R]
        av_f = folded[0][4]
        hn0_f32 = hn0_ds.astype(FP32)
        m["aqx"] = np.ascontiguousarray(aq_f.T @ hn0_f32).astype(BF16)  # [R,S]
        m["avx"] = np.ascontiguousarray(av_f.T @ hn0_f32).astype(BF16)
        # attention_mask bias [128, TC]: col j, part p -> key token 128j+p
        mb = np.where(amask[b] > 0, FP32(0), FP32(-1e9)).reshape(TC, 128).T
        m["maskbias"] = np.ascontiguousarray(mb)
        m["mbtail"] = np.ascontiguousarray(np.tile(mb, (1, NH)))
        for l in range(L):
            wq, wk, wv, aq, av, bq, bv, wo, wg, wu, wd = folded[l]
            dsl = slice(r * DL, (r + 1) * DL)
            fsl = slice(r * FL, (r + 1) * FL)
            m[f"wq{l}"] = np.ascontiguousarray(
                wq[:, dsl].reshape(KC, 128, DL).transpose(1, 0, 2)).astype(BF16)
            m[f"wk{l}"] = np.ascontiguousarray(
                wk[:, dsl].reshape(KC, 128, DL).transpose(1, 0, 2)).astype(BF16)
            m[f"wv{l}"] = np.ascontiguousarray(
                wv[:, dsl].reshape(KC, 128, DL).transpose(1, 0, 2)).astype(BF16)
            m[f"aq{l}"] = np.ascontiguousarray(aq.reshape(KC, 128, R)).astype(BF16)
            m[f"av{l}"] = np.ascontiguousarray(av.reshape(KC, 128, R)).astype(BF16)
            m[f"bq{l}"] = np.ascontiguousarray(bq[:, dsl]).astype(BF16)
            m[f"bv{l}"] = np.ascontiguousarray(bv[:, dsl]).astype(BF16)
            m[f"wo{l}"] = np.ascontiguousarray(wo[dsl].reshape(NH, 128, D)).astype(BF16)
            # wg/wu: [D, FL] -> [FC, 128(d-part), KC, 128(f-col)] so the
            # per-fc SBUF load [128, KC, 128] is a contiguous DMA
            wgl = wg[:, fsl].reshape(KC, 128, FC, 128).transpose(2, 1, 0, 3)
            wul = wu[:, fsl].reshape(KC, 128, FC, 128).transpose(2, 1, 0, 3)
            m[f"wg{l}"] = np.ascontiguousarray(wgl).astype(BF16)
            m[f"wu{l}"] = np.ascontiguousarray(wul).astype(BF16)
            m[f"wd{l}"] = np.ascontiguousarray(wd[fsl].reshape(FC, 128, D)).astype(BF16)
        in_maps.append(m)
    return in_maps


def run_on_device(in_maps, trace=False):
    nc = _get_program()
    return bass_utils.run_bass_kernel_spmd(
        nc, in_maps, core_ids=list(range(N_CORES)), trace=trace,
    )


def kernel(**inputs):
    in_maps = _host_prepare(inputs)
    res = run_on_device(in_maps, trace=False)
    out = np.stack([
        res.results[0]["out"].reshape(OUT),
        res.results[TP]["out"].reshape(OUT),
    ]).astype(FP32)
    return out



# revision 54
# speedup vs baseline: 5.1194x; 5.1194x over previous
"""Trainium2 Bass kernel for nn_LlamaForSequenceRegression_14336600834254.

2-layer Llama (D=2048, H=16, HD=128, F=5632, LoRA r=16 on q/v) + regression
head, B=2, S=1024, fp32 reference.

Distribution (8 NeuronCores): DP2 x TP4.
  - cores 0-3 process batch 0, cores 4-7 batch 1 (data parallel).
  - within each group of 4: Megatron tensor parallel - Wq/Wk/Wv column
    shards (4 heads/core), Wo row shards, Wgate/Wup column shards
    (F/4=1408), Wdown row shards. AllReduce (bf16) after attn-out and
    after MLP-down, replica_groups=[[0,1,2,3],[4,5,6,7]].
  - embedding gather, the layer-0 input rmsnorm, the layer-0 LoRA
    down-projections, and norm-weight folding are done host-side; all
    device matmuls run in bf16 with fp32 PSUM accumulation; the
    residual stream / softmax / rmsnorm statistics are fp32.

Performance structure (vs the first working version):
  - all weights stored host-side in partition-major layouts so every
    SBUF load is a large contiguous DMA (k-pair / og-pair tiles).
  - layer-0 norm + LoRA down-activations computed on host; hn0 uploads
    gate the first projections, bulk h upload is deferred.
  - bf16 residual stream (validated: adds ~2e-3 rel err).
  - per-half pipeline: q-proj/rope interleaved per head with attention
    so the out-proj AllReduce of half 0 starts as early as possible;
    MLP AllReduces per half overlap the next phase.
  - last-token tail: wq preloaded into SBUF during layer 0; the tail
    q/attention/out-proj/MLP use column-packed PSUM accumulation (one
    has_written group per bank) with one activation op per stage; tail
    MLP weights are preloaded ahead of the AllReduce so their DMAs are
    not FIFO-blocked behind the AR-landing descriptor.
"""

import numpy as np
import ml_dtypes

import concourse.bacc as bacc
import concourse.tile as tile
from concourse import mybir
from concourse import bass_utils

BF16 = ml_dtypes.bfloat16
FP32 = np.float32

V, D, L, H, HD, F, R, ALPHA, B, S, OUT = 32000, 2048, 2, 16, 128, 5632, 16, 32, 2, 1024, 11
EPS = 1e-5
SCALING = ALPHA / R
N_CORES = 8
TP = 4
NH = H // TP          # 4 local heads
DL = NH * HD          # 512 local q/k/v cols
FL = F // TP          # 1408 local mlp cols
KC = D // 128         # 16 contraction chunks
FC = FL // 128        # 11 mlp chunks
TT = 512              # token tile (free dim per matmul)
NT = S // TT          # 2 token tiles
TC = S // 128         # 8 token chunks (128-wide)
OW = TT // TP         # 128 tokens owned per rank per half (seq-parallel)
REPLICA_GROUPS = [[0, 1, 2, 3], [4, 5, 6, 7]]

dt = mybir.dt


def build_program(reps=1, out_groups=4, wo_q="sp", wscale=1.0):
    """Build the SPMD Bass program (identical on all 8 cores; weights differ
    per core via the input maps)."""
    nc = bacc.Bacc(num_devices=N_CORES, debug=False)

    # ---- DRAM I/O ----
    # xT: only the rank-owned token chunks (seq-parallel residual)
    xT = nc.dram_tensor("xT", [128, KC, NT * OW], dt.bfloat16, kind="ExternalInput")
    hsel = nc.dram_tensor("hsel", [128, 1], dt.bfloat16, kind="ExternalInput")
    hn0 = nc.dram_tensor("hn0", [128, KC, S], dt.bfloat16, kind="ExternalInput")
    aqx = nc.dram_tensor("aqx", [R, S], dt.bfloat16, kind="ExternalInput")
    avx = nc.dram_tensor("avx", [R, S], dt.bfloat16, kind="ExternalInput")
    cosT = nc.dram_tensor("cosT", [128, S], dt.bfloat16, kind="ExternalInput")
    sinT = nc.dram_tensor("sinT", [128, S], dt.bfloat16, kind="ExternalInput")
    mstrip = nc.dram_tensor("mstrip", [128, 896], dt.bfloat16, kind="ExternalInput")
    maskbias = nc.dram_tensor("maskbias", [128, TC], dt.float32, kind="ExternalInput")
    mbtail = nc.dram_tensor("mbtail", [128, NH * TC], dt.float32, kind="ExternalInput")
    wreg = nc.dram_tensor("wreg", [128, KC, OUT], dt.bfloat16, kind="ExternalInput")
    breg = nc.dram_tensor("breg", [OUT, 1], dt.float32, kind="ExternalInput")
    W = {}
    for l in range(L):
        W[f"wq{l}"] = nc.dram_tensor(f"wq{l}", [128, KC, DL], dt.bfloat16, kind="ExternalInput")
        W[f"wk{l}"] = nc.dram_tensor(f"wk{l}", [128, KC, DL], dt.bfloat16, kind="ExternalInput")
        W[f"wv{l}"] = nc.dram_tensor(f"wv{l}", [128, KC, DL], dt.bfloat16, kind="ExternalInput")
        if l > 0:
            W[f"aq{l}"] = nc.dram_tensor(f"aq{l}", [KC, 128, R], dt.bfloat16, kind="ExternalInput")
            W[f"av{l}"] = nc.dram_tensor(f"av{l}", [KC, 128, R], dt.bfloat16, kind="ExternalInput")
        W[f"bq{l}"] = nc.dram_tensor(f"bq{l}", [R, DL], dt.bfloat16, kind="ExternalInput")
        W[f"bv{l}"] = nc.dram_tensor(f"bv{l}", [R, DL], dt.bfloat16, kind="ExternalInput")
        W[f"wo{l}"] = nc.dram_tensor(f"wo{l}", [NH, 128, D], dt.bfloat16, kind="ExternalInput")
        # [FC, 128(d-part), KC, 128(f-col)] -> contiguous per-fc SBUF loads
        W[f"wg{l}"] = nc.dram_tensor(f"wg{l}", [FC, 128, KC, 128], dt.bfloat16, kind="ExternalInput")
        W[f"wu{l}"] = nc.dram_tensor(f"wu{l}", [FC, 128, KC, 128], dt.bfloat16, kind="ExternalInput")
        W[f"wd{l}"] = nc.dram_tensor(f"wd{l}", [FC, 128, D], dt.bfloat16, kind="ExternalInput")
    out_dram = nc.dram_tensor("out", [OUT, 1], dt.float32, kind="ExternalOutput")

    with tile.TileContext(nc) as tc:
        with (
            tc.tile_pool(name="persist", bufs=1) as pp,
            tc.tile_pool(name="wts", bufs=4) as wp,
            tc.tile_pool(name="colw", bufs=5) as cwp,
            tc.tile_pool(name="tmp", bufs=3) as tp_,
            tc.tile_pool(name="stage", bufs=2) as stp,
            tc.tile_pool(name="psum", bufs=8, space="PSUM") as ps,
            tc.tile_pool(name="dram", bufs=1, space="DRAM") as dram,
        ):
            f32, bf = dt.float32, dt.bfloat16
            # ---- persistent tiles ----
            # hres: the rank's OWNED residual tokens only ([half t, slot w]
            # = global token t*TT + r*OW + w for TP rank r).
            hres = pp.tile([128, KC, NT * OW], bf, tag="hres")
            hsel_sb = pp.tile([128, 1], bf, tag="hsel")
            hn = pp.tile([128, KC, S], bf, tag="hn")
            cos_sb = pp.tile([128, S], bf, tag="cos")
            sin_sb = pp.tile([128, S], bf, tag="sin")
            mstrip_sb = pp.tile([128, 896], bf, tag="mstrip")
            mb_sb = pp.tile([128, TC], f32, tag="mb")
            mbt_sb = pp.tile([128, NH * TC], f32, tag="mbt")
            oneD_sb = pp.tile([128, 1], bf, tag="oneD")
            ones_bf = pp.tile([128, 1], bf, tag="onesbf")
            eps_sb = pp.tile([1, 1], f32, tag="eps")
            ones_row = pp.tile([1, 128], f32, tag="ones_row")
            qT = pp.tile([128, NH, S], bf, tag="qT")
            kT = pp.tile([128, NH, S], bf, tag="kT")
            vN = pp.tile([128, TC, DL], bf, tag="vN")
            ctxT = pp.tile([128, NH, S], bf, tag="ctxT")
            mT = pp.tile([128, FC, S], bf, tag="mT")
            aqw = pp.tile([128, KC, R], bf, tag="aqw")
            avw = pp.tile([128, KC, R], bf, tag="avw")
            bq_sb = pp.tile([R, DL], bf, tag="bq")
            bv_sb = pp.tile([R, DL], bf, tag="bv")
            aq_sb = pp.tile([R, S], bf, tag="aq")
            av_sb = pp.tile([R, S], bf, tag="av")
            wreg_sb = pp.tile([128, KC, OUT], bf, tag="wreg")
            breg_sb = pp.tile([OUT, 1], f32, tag="breg")
            wq1_sb = pp.tile([128, KC, DL], bf, tag="wq1")

            for _rep in range(reps):
                # ---- constants in ----
                # hn0 first: it gates the first projections
                for k0, k1 in ((0, 2), (2, 4), (4, 8), (8, 12), (12, 16)):
                    nc.sync.dma_start(hn[:, k0:k1, :], hn0[:, k0:k1, :])
                nc.sync.dma_start(cos_sb[:], cosT[:])
                nc.sync.dma_start(sin_sb[:], sinT[:])
                nc.vector.memset(oneD_sb[:], 1.0 / D)
                nc.vector.memset(ones_bf[:], 1.0)
                nc.vector.memset(eps_sb[:], EPS)
                nc.vector.memset(ones_row[:], 1.0)
                nc.sync.dma_start(aq_sb[:], aqx[:])
                nc.sync.dma_start(av_sb[:], avx[:])
                nc.sync.dma_start(hsel_sb[:], hsel[:])

                # DRAM bounce buffers for collectives.  Boundary = partial
                # sums staged rank-major [TP, 128, KC, OW] per half ->
                # ReduceScatter (rank gets its OW-token chunk) -> local
                # add+rmsnorm -> AllGather normed full half back into hn.
                # (HW collectives require contiguous APs, so the staging DMA
                # writes the rank-major layout directly.)
                ar1h_in = [dram.tile([TP, 128, KC, OW], bf, name=f"ar1hi_{t}") for t in range(NT)]
                ar2h_in = [dram.tile([TP, 128, KC, OW], bf, name=f"ar2hi_{t}") for t in range(NT)]
                rs_out = {(b, t): dram.tile([128, KC, OW], bf, name=f"rso_{b}_{t}")
                          for b in (1, 2) for t in range(NT)}
                ag_in = {(b, t): dram.tile([128, KC, OW], bf, name=f"agi_{b}_{t}")
                         for b in (1, 2) for t in range(NT)}
                ag_out = {(b, t): dram.tile([TP, 128, KC, OW], bf, name=f"ago_{b}_{t}")
                          for b in (1, 2) for t in range(NT)}

                def stage_rank_major(dst, k, pso):
                    """PSUM [128, TT] f32 partial -> bf16 stage tile (DVE
                    copy: the Act queue is contended by softmax exps) ->
                    dst[TP, 128, k, OW] rank-major for ReduceScatter."""
                    st = stp.tile([128, TT], bf, tag="stage")
                    nc.vector.tensor_mul(st[:], pso[:],
                                         ones_bf[:].broadcast_to([128, TT]))
                    # rearrange the DRAM side only: SBUF APs must keep the
                    # partition dim leading
                    nc.sync.dma_start(
                        dst[:, :, k, :].transpose([1, 0, 2]),
                        st[:].rearrange("p (r w) -> p r w", r=TP))

                def rs_half(b, t):
                    src = (ar1h_in if b == 1 else ar2h_in)[t]
                    nc.gpsimd.collective_compute(
                        "ReduceScatter", mybir.AluOpType.add,
                        replica_groups=REPLICA_GROUPS,
                        ins=[src.opt()], outs=[rs_out[b, t].opt()],
                    )

                def ag_half(b, t):
                    nc.gpsimd.collective_compute(
                        "AllGather", mybir.AluOpType.bypass,
                        replica_groups=REPLICA_GROUPS,
                        ins=[ag_in[b, t].opt()], outs=[ag_out[b, t].opt()],
                    )

                def boundary_stage(b, t):
                    """Land the rank's summed OW-token chunk, add into hres,
                    rmsnorm it, stage the normed chunk for AllGather.
                    Elementwise work runs on gpsimd (Pool) — the engine is
                    otherwise idle and its queue is where the collectives
                    live, so the ops slot naturally between RS and AG."""
                    cs = slice(t * OW, (t + 1) * OW)
                    lb = tp_.tile([128, KC, OW], bf, tag="rsland", bufs=2,
                                  name=f"rsl_{b}_{t}")
                    nc.sync.dma_start(lb[:], rs_out[b, t][:])
                    nc.gpsimd.tensor_add(hres[:, :, cs], hres[:, :, cs], lb[:])
                    sumsq = ps.tile([128, OW], f32, tag="psum", name=f"nsum_{b}_{t}")
                    for k in range(KC):
                        sq = tp_.tile([128, OW], bf, tag="sqc", bufs=2,
                                      name=f"nsq_{b}_{t}_{k}")
                        nc.gpsimd.tensor_mul(sq[:], hres[:, k, cs], hres[:, k, cs])
                        nc.tensor.matmul(sumsq[0:1, :], oneD_sb[:], sq[:],
                                         start=(k == 0), stop=(k == KC - 1))
                    rsl = tp_.tile([1, OW], f32, tag="vec1c", bufs=2,
                                   name=f"rslv_{b}_{t}")
                    nc.scalar.activation(rsl[0:1, :], sumsq[0:1, :],
                                         mybir.ActivationFunctionType.Sqrt, bias=eps_sb[:])
                    nc.vector.reciprocal(rsl[0:1, :], rsl[0:1, :])
                    psrs = ps.tile([128, OW], f32, tag="psum", name=f"psrs_{b}_{t}")
                    nc.tensor.matmul(psrs[:], ones_row[:], rsl[0:1, :],
                                     start=True, stop=True)
                    # gpsimd cannot read PSUM - bounce through SBUF
                    rs_sb = tp_.tile([128, OW], f32, tag="rs_sb", bufs=2,
                                     name=f"rssb_{b}_{t}")
                    nc.scalar.copy(rs_sb[:], psrs[:])
                    hnc = tp_.tile([128, KC, OW], bf, tag="hnc", bufs=2,
                                   name=f"hnc_{b}_{t}")
                    for k in range(KC):
                        nc.gpsimd.tensor_mul(hnc[:, k, :], hres[:, k, cs], rs_sb[:])
                    nc.sync.dma_start(ag_in[b, t][:], hnc[:])

                def land_hn(b, t):
                    for r in range(TP):
                        rs_ = slice(t * TT + r * OW, t * TT + (r + 1) * OW)
                        nc.sync.dma_start(hn[:, :, rs_], ag_out[b, t][r])

                def rope_from_psum(psq, dst, hc, t):
                    """Apply RoPE to psum [128,TT] (one head, token tile t) and
                    write bf16 into dst[:, hc, t*TT:...]."""
                    ts_ = slice(t * TT, (t + 1) * TT)
                    # cos/sin tiles carry the same 64-row table duplicated into
                    # both partition halves so every SB+SB operand pair below is
                    # base-partition aligned.
                    t2 = tp_.tile([128, TT], bf, tag="ropetB", bufs=2)
                    t4 = tp_.tile([128, TT], bf, tag="ropetB", bufs=2)
                    nc.vector.tensor_mul(dst[0:64, hc, ts_], psq[0:64, :], cos_sb[0:64, ts_])
                    nc.vector.tensor_mul(t2[0:64, :], psq[64:128, :], sin_sb[0:64, ts_])
                    nc.vector.tensor_sub(dst[0:64, hc, ts_], dst[0:64, hc, ts_], t2[0:64, :])
                    nc.vector.tensor_mul(dst[64:128, hc, ts_], psq[64:128, :], cos_sb[64:128, ts_])
                    nc.vector.tensor_mul(t4[64:128, :], psq[0:64, :], sin_sb[64:128, ts_])
                    nc.vector.tensor_add(dst[64:128, hc, ts_], dst[64:128, hc, ts_], t4[64:128, :])

                def qk_attn_out_half(l, t):
                    """q-proj for half t, then attention per head with score
                    matmuls batched in groups (separate PSUM banks) so the
                    PE->exp->accumulate chain pipelines instead of
                    serializing per key-chunk."""
                    ts_ = slice(t * TT, (t + 1) * TT)
                    jmax = (t + 1) * (TT // 128)
                    psq = [ps.tile([128, TT], f32, tag="psum", name=f"psqf_{hc}_{t}")
                           for hc in range(NH)]
                    for k2 in range(KC // 2):
                        wt = wp.tile([128, 2, DL], bf, tag="wqkv", bufs=3,
                                     name=f"wqf_{k2}_{t}")
                        nc.sync.dma_start(wt[:], W[f"wq{l}"][:, 2 * k2:2 * k2 + 2, :])
                        for kk in range(2):
                            k = 2 * k2 + kk
                            for hc in range(NH):
                                nc.tensor.matmul(
                                    psq[hc][:], wt[:, kk, hc * HD:(hc + 1) * HD],
                                    hn[:, k, ts_], start=(k == 0), stop=False,
                                )
                    # rope all heads first: frees the psq banks before the
                    # score groups need them
                    for hh in range(NH):
                        nc.tensor.matmul(
                            psq[hh][:], bq_sb[:, hh * HD:(hh + 1) * HD],
                            aq_sb[:, ts_], start=False, stop=True,
                        )
                        rope_from_psum(psq[hh], qT, hh, t)
                    G = 2
                    for hh in range(NH):
                        psd = ps.tile([128, TT], f32, tag="psum")
                        psc = ps.tile([128, TT], f32, tag="psum")
                        groups = [list(range(j0, min(j0 + G, jmax)))
                                  for j0 in range(0, jmax, G)]
                        sc, exs = {}, {}

                        def scores(g):
                            for j in groups[g]:
                                pss = ps.tile([128, TT], f32, tag="psum",
                                              name=f"pss_{t}_{hh}_{j}")
                                nc.tensor.matmul(
                                    pss[:], kT[:, hh, j * 128:(j + 1) * 128],
                                    qT[:, hh, ts_], start=True, stop=True,
                                )
                                sc[j] = pss

                        def exps(g):
                            for j in groups[g]:
                                ex = tp_.tile([128, TT], bf, tag="exj", bufs=4,
                                              name=f"ex_{t}_{hh}_{j}")
                                nc.scalar.activation(
                                    ex[:], sc[j][:],
                                    mybir.ActivationFunctionType.Exp,
                                    bias=mb_sb[:, j:j + 1], scale=1.0,
                                )
                                off = t * TT - j * 128
                                if off < 128:
                                    # diagonal tile: multiply 0/1 causal strip
                                    # allowed iff p <= f + off
                                    nc.vector.tensor_mul(
                                        ex[:], ex[:],
                                        mstrip_sb[:, 384 + off:896 + off],
                                    )
                                exs[j] = ex

                        def accums(g):
                            for j in groups[g]:
                                nc.tensor.matmul(
                                    psd[0:1, :], ones_bf[:], exs[j][:],
                                    start=(j == 0), stop=(j == jmax - 1),
                                )
                                nc.tensor.matmul(
                                    psc[:], vN[:, j, hh * HD:(hh + 1) * HD],
                                    exs[j][:],
                                    start=(j == 0), stop=(j == jmax - 1),
                                )

                        scores(0)
                        for g in range(len(groups)):
                            exps(g)
                            if g + 1 < len(groups):
                                scores(g + 1)
                            accums(g)
                        rdl = tp_.tile([1, TT], f32, tag="vec1", bufs=2,
                                       name=f"rdl_{t}_{hh}")
                        nc.vector.reciprocal(rdl[0:1, :], psd[0:1, :])
                        # broadcast 1/denom over partitions via PE outer
                        # product: keeps the Pool queue free for collectives
                        psrd = ps.tile([128, TT], f32, tag="psum",
                                       name=f"psrd_{t}_{hh}")
                        nc.tensor.matmul(psrd[:], ones_row[:], rdl[0:1, :],
                                         start=True, stop=True)
                        # HW: only one PSUM operand per vector op - bounce
                        # the broadcast through SBUF
                        rden_sb = tp_.tile([128, TT], f32, tag="rden", bufs=2,
                                           name=f"rden_{t}_{hh}")
                        nc.scalar.copy(rden_sb[:], psrd[:])
                        nc.vector.tensor_mul(ctxT[:, hh, ts_], psc[:], rden_sb[:])
                    out_proj_half(l, t)

                def out_proj_half(l, t):
                    """attn partial for token half t -> ar1h_in[t]."""
                    go = out_groups
                    wo_eng = {"sp": nc.sync, "act": nc.scalar,
                              "pool": nc.gpsimd}[wo_q]
                    for og in range(16 // go):
                        pso = [ps.tile([128, TT], f32, tag="psum",
                                       name=f"pso_{og}_{oi}_{t}")
                               for oi in range(go)]
                        for hc in range(NH):
                            wt = wp.tile([128, go * 128], bf, tag="wo", bufs=4,
                                         name=f"wo_t_{og}_{hc}_{t}")
                            wo_eng.dma_start(
                                wt[:], W[f"wo{l}"][hc][:, og * go * 128:(og + 1) * go * 128])
                            for oi in range(go):
                                nc.tensor.matmul(
                                    pso[oi][:],
                                    wt[:, oi * 128:(oi + 1) * 128],
                                    ctxT[:, hc, t * TT:(t + 1) * TT],
                                    start=(hc == 0), stop=(hc == NH - 1),
                                )
                        for oi in range(go):
                            stage_rank_major(ar1h_in[t], og * go + oi, pso[oi])

                def lora_down_half(aw, dst, t):
                    psa = ps.tile([128, TT], f32, tag="psum", name=f"psld_{t}")
                    for k in range(KC):
                        nc.tensor.matmul(
                            psa[0:R, :], aw[:, k, :], hn[:, k, t * TT:(t + 1) * TT],
                            start=(k == 0), stop=(k == KC - 1),
                        )
                    nc.scalar.copy(dst[:, t * TT:(t + 1) * TT], psa[0:R, :])

                def k_proj_half(wname, dst, t):
                    psq = [ps.tile([128, TT], f32, tag="psum", name=f"psqh_{hc}_{t}")
                           for hc in range(NH)]
                    for k2 in range(KC // 2):
                        wt = wp.tile([128, 2, DL], bf, tag="wqkv", bufs=3,
                                     name=f"wkh_{k2}_{t}")
                        nc.sync.dma_start(wt[:], W[wname][:, 2 * k2:2 * k2 + 2, :])
                        for kk in range(2):
                            k = 2 * k2 + kk
                            for hc in range(NH):
                                nc.tensor.matmul(
                                    psq[hc][:], wt[:, kk, hc * HD:(hc + 1) * HD],
                                    hn[:, k, t * TT:(t + 1) * TT],
                                    start=(k == 0), stop=(k == KC - 1),
                                )
                    for hc in range(NH):
                        rope_from_psum(psq[hc], dst, hc, t)

                def v_proj_half(l, t):
                    psv = [ps.tile([128, DL], f32, tag="psum", name=f"psvh_{c}_{t}")
                           for c in range(4)]
                    for k2 in range(KC // 2):
                        wt = wp.tile([128, 2, DL], bf, tag="wqkv", bufs=3,
                                     name=f"wvh_{k2}_{t}")
                        nc.sync.dma_start(wt[:], W[f"wv{l}"][:, 2 * k2:2 * k2 + 2, :])
                        for kk in range(2):
                            k = 2 * k2 + kk
                            for ci in range(4):
                                c = t * 4 + ci
                                nc.tensor.matmul(
                                    psv[ci][:], hn[:, k, c * 128:(c + 1) * 128],
                                    wt[:, kk, :],
                                    start=(k == 0), stop=False,
                                )
                    for ci in range(4):
                        c = t * 4 + ci
                        nc.tensor.matmul(
                            psv[ci][:], av_sb[:, c * 128:(c + 1) * 128], bv_sb[:],
                            start=False, stop=True,
                        )
                        nc.scalar.copy(vN[:, c, :], psv[ci][:])

                def mlp_gate_up_half(l, t, fcs=None):
                    ts_ = slice(t * TT, (t + 1) * TT)
                    for fc in (range(FC) if fcs is None else fcs):
                        wg_h = [cwp.tile([128, KC // 2, 128], bf, tag="wgcol",
                                         name=f"wg_{fc}_{t}_{hh}") for hh in range(2)]
                        wu_h = [cwp.tile([128, KC // 2, 128], bf, tag="wucol",
                                         name=f"wu_{fc}_{t}_{hh}") for hh in range(2)]
                        for hh in range(2):
                            ksl = slice(hh * (KC // 2), (hh + 1) * (KC // 2))
                            nc.sync.dma_start(wg_h[hh][:], W[f"wg{l}"][fc][:, ksl, :])
                            nc.sync.dma_start(wu_h[hh][:], W[f"wu{l}"][fc][:, ksl, :])
                        psg = ps.tile([128, TT], f32, tag="psum", name=f"psg_{fc}_{t}")
                        psu = ps.tile([128, TT], f32, tag="psum", name=f"psu_{fc}_{t}")
                        for k in range(KC):
                            nc.tensor.matmul(psg[:], wg_h[k // 8][:, k % 8, :],
                                             hn[:, k, ts_],
                                             start=(k == 0), stop=(k == KC - 1))
                            nc.tensor.matmul(psu[:], wu_h[k // 8][:, k % 8, :],
                                             hn[:, k, ts_],
                                             start=(k == 0), stop=(k == KC - 1))
                        sg = tp_.tile([128, TT], bf, tag="silu", bufs=2, name=f"sg_{fc}_{t}")
                        nc.scalar.activation(sg[:], psg[:], mybir.ActivationFunctionType.Silu)
                        nc.vector.tensor_mul(mT[:, fc, ts_], sg[:], psu[:])

                def mlp_down_half(l, t):
                    ts_ = slice(t * TT, (t + 1) * TT)
                    for og in range(4):
                        pso = [ps.tile([128, TT], f32, tag="psum",
                                       name=f"psd_{og}_{oi}_{t}")
                               for oi in range(4)]
                        for kc in range(FC):
                            wt = wp.tile([128, TT], bf, tag="wd", bufs=4,
                                         name=f"wd_t_{og}_{kc}_{t}")
                            nc.sync.dma_start(
                                wt[:], W[f"wd{l}"][kc][:, og * 512:(og + 1) * 512])
                            for oi in range(4):
                                nc.tensor.matmul(
                                    pso[oi][:],
                                    wt[:, oi * 128:(oi + 1) * 128],
                                    mT[:, kc, ts_],
                                    start=(kc == 0), stop=(kc == FC - 1),
                                )
                        for oi in range(4):
                            stage_rank_major(ar2h_in[t], og * 4 + oi, pso[oi])

                # ---------- last-token ("tail") variants for the final layer ----
                # The residual at token S-1 lives only on TP-rank 3 (its last
                # owned slot).  hsel (1.0 on rank 3, else 0) folds it into the
                # first tail AllReduce so every rank gets h_last + attn_sum.
                ar_in_s = dram.tile([128, KC, 1], bf)
                ar_out_s = dram.tile([128, KC, 1], bf)
                ar_in_s2 = dram.tile([128, KC, 1], bf)
                ar_out_s2 = dram.tile([128, KC, 1], bf)
                q_last = pp.tile([128, NH], bf, tag="q_last")
                ctx_last = pp.tile([128, NH], bf, tag="ctx_last")
                hn_last = pp.tile([128, KC, 1], bf, tag="hn_last")
                h_last1 = pp.tile([128, KC, 1], bf, tag="h_last1")
                m_last = pp.tile([128, FC], bf, tag="m_last")
                exps_tail = pp.tile([128, NH * TC], bf, tag="exps_tail")

                def q_proj_tail(l):
                    psq = ps.tile([128, TT], f32, tag="psum", name="psqt")
                    for k in range(KC):
                        for hc in range(NH):
                            # one has_written group for the whole bank: start
                            # clears the full bank, so only the very first
                            # matmul may set it (per-element overwrite covers
                            # each column's first write)
                            nc.tensor.matmul(
                                psq[:, hc:hc + 1],
                                wq1_sb[:, k, hc * HD:(hc + 1) * HD],
                                hn[:, k, S - 1:S],
                                start=(k == 0 and hc == 0), stop=False,
                            )
                    for hc in range(NH):
                        nc.tensor.matmul(
                            psq[:, hc:hc + 1], bq_sb[:, hc * HD:(hc + 1) * HD],
                            aq_sb[:, S - 1:S], start=False, stop=(hc == NH - 1),
                        )
                    # batched rope over all NH columns
                    t2 = tp_.tile([128, NH], f32, tag="ropetS", bufs=2)
                    t4 = tp_.tile([128, NH], f32, tag="ropetS", bufs=2)
                    cb_lo = cos_sb[0:64, S - 1:S].broadcast_to([64, NH])
                    cb_hi = cos_sb[64:128, S - 1:S].broadcast_to([64, NH])
                    sb_lo = sin_sb[0:64, S - 1:S].broadcast_to([64, NH])
                    sb_hi = sin_sb[64:128, S - 1:S].broadcast_to([64, NH])
                    nc.vector.tensor_mul(q_last[0:64, :], psq[0:64, 0:NH], cb_lo)
                    nc.vector.tensor_mul(t2[0:64, :], psq[64:128, 0:NH], sb_lo)
                    nc.vector.tensor_sub(q_last[0:64, :], q_last[0:64, :], t2[0:64, :])
                    nc.vector.tensor_mul(q_last[64:128, :], psq[64:128, 0:NH], cb_hi)
                    nc.vector.tensor_mul(t4[64:128, :], psq[0:64, 0:NH], sb_hi)
                    nc.vector.tensor_add(q_last[64:128, :], q_last[64:128, :], t4[64:128, :])

                def attention_tail():
                    # last token attends to every key: no causal strip needed.
                    # scores for all heads land as columns [keys-part, hh*TC+j].
                    pss = ps.tile([128, TT], f32, tag="psum", name="psst")
                    for hh in range(NH):
                        for j in range(TC):
                            nc.tensor.matmul(
                                pss[:, hh * TC + j:hh * TC + j + 1],
                                kT[:, hh, j * 128:(j + 1) * 128],
                                q_last[:, hh:hh + 1],
                                start=(hh == 0 and j == 0),
                                stop=(hh == NH - 1 and j == TC - 1),
                            )
                    # attention-mask bias per key (zeros for an all-ones mask)
                    nc.vector.tensor_add(pss[:, 0:NH * TC], pss[:, 0:NH * TC],
                                         mbt_sb[:])
                    nc.scalar.activation(exps_tail[:], pss[:, 0:NH * TC],
                                         mybir.ActivationFunctionType.Exp)
                    psd = ps.tile([128, TT], f32, tag="psum", name="psdt")
                    psc = ps.tile([128, TT], f32, tag="psum", name="psct")
                    for hh in range(NH):
                        for j in range(TC):
                            first = hh == 0 and j == 0
                            last = hh == NH - 1 and j == TC - 1
                            nc.tensor.matmul(psd[0:1, hh:hh + 1], ones_bf[:],
                                             exps_tail[:, hh * TC + j:hh * TC + j + 1],
                                             start=first, stop=last)
                            nc.tensor.matmul(psc[:, hh:hh + 1],
                                             vN[:, j, hh * HD:(hh + 1) * HD],
                                             exps_tail[:, hh * TC + j:hh * TC + j + 1],
                                             start=first, stop=last)
                    rd = tp_.tile([1, NH], f32, tag="rd_tail", bufs=2)
                    nc.vector.reciprocal(rd[:], psd[0:1, 0:NH])
                    rdb = tp_.tile([128, NH], f32, tag="rdb_tail", bufs=2)
                    nc.gpsimd.partition_broadcast(rdb[:], rd[:])
                    nc.vector.tensor_mul(ctx_last[:], psc[:, 0:NH], rdb[:])

                def out_proj_tail(l):
                    pso = ps.tile([128, KC], f32, tag="psum", name="psot")
                    for hc in range(NH):
                        wt = wp.tile([128, D], bf, tag="wdtail", bufs=3,
                                     name=f"wot_{hc}")
                        nc.sync.dma_start(wt[:], W[f"wo{l}"][hc])
                        for oc in range(KC):
                            nc.tensor.matmul(
                                pso[:, oc:oc + 1],
                                wt[:, oc * 128:(oc + 1) * 128],
                                ctx_last[:, hc:hc + 1],
                                start=(hc == 0 and oc == 0),
                                stop=(hc == NH - 1 and oc == KC - 1),
                            )
                    st = stp.tile([128, KC], bf, tag="stage_tail", bufs=2)
                    nc.scalar.copy(st[:], pso[:, 0:KC])
                    # + hsel * h_last (residual contributed by rank 3 only)
                    hl = tp_.tile([128, KC], bf, tag="hseltail")
                    nc.vector.tensor_mul(hl[:], hres[:, :, NT * OW - 1],
                                         hsel_sb[:].broadcast_to([128, KC]))
                    nc.vector.tensor_add(st[:], st[:], hl[:])
                    nc.sync.dma_start(ar_in_s[:, :, 0], st[:])

                def ar_tail(ain, aout, dst):
                    nc.gpsimd.collective_compute(
                        "AllReduce", mybir.AluOpType.add,
                        replica_groups=REPLICA_GROUPS,
                        ins=[ain.opt()], outs=[aout.opt()],
                    )
                    nc.sync.dma_start(dst[:], aout[:])

                def norm_tail_to(src, dst):
                    psl = ps.tile([128, TT], f32, tag="psum", name="psl_norm_tail")
                    sqt = tp_.tile([128, KC, 1], bf, tag="sqlast")
                    nc.scalar.activation(sqt[:], src[:],
                                         mybir.ActivationFunctionType.Square)
                    for k in range(KC):
                        nc.tensor.matmul(psl[0:1, 0:1], oneD_sb[:], sqt[:, k, :],
                                         start=(k == 0), stop=(k == KC - 1))
                    rst = tp_.tile([1, 1], f32, tag="rst_tail")
                    nc.scalar.activation(rst[:], psl[0:1, 0:1],
                                         mybir.ActivationFunctionType.Sqrt, bias=eps_sb[:])
                    nc.vector.reciprocal(rst[:], rst[:])
                    rstb = tp_.tile([128, 1], f32, tag="rstb_tail")
                    nc.gpsimd.partition_broadcast(rstb[:], rst[:])
                    nc.vector.tensor_mul(
                        dst[:], src[:],
                        rstb[:, :, None].broadcast_to([128, 1, 1]).broadcast_to([128, KC, 1]),
                    )

                def mlp_tail_preload(l, nfc=4, nkc=2):
                    pre = {}
                    for fc in range(nfc):
                        wg_h = [cwp.tile([128, KC // 2, 128], bf, tag="wgcol",
                                         name=f"wgt_{fc}_{hh}") for hh in range(2)]
                        wu_h = [cwp.tile([128, KC // 2, 128], bf, tag="wucol",
                                         name=f"wut_{fc}_{hh}") for hh in range(2)]
                        for hh in range(2):
                            ksl = slice(hh * (KC // 2), (hh + 1) * (KC // 2))
                            nc.sync.dma_start(wg_h[hh][:], W[f"wg{l}"][fc][:, ksl, :])
                            nc.sync.dma_start(wu_h[hh][:], W[f"wu{l}"][fc][:, ksl, :])
                        pre[fc] = (wg_h, wu_h)
                    pre["wd"] = []
                    for kc in range(nkc):
                        wt = wp.tile([128, D], bf, tag="wdtail", bufs=3,
                                     name=f"wdtp_{kc}")
                        nc.sync.dma_start(wt[:], W[f"wd{l}"][kc])
                        pre["wd"].append(wt)
                    return pre

                def mlp_tail(l, pre):
                    norm_tail_to(h_last1, hn_last)
                    psg = ps.tile([128, TT], f32, tag="psum", name="psgt")
                    psu = ps.tile([128, TT], f32, tag="psum", name="psut")
                    for fc in range(FC):
                        if fc in pre:
                            wg_h, wu_h = pre[fc]
                        else:
                            wg_h = [cwp.tile([128, KC // 2, 128], bf, tag="wgcol",
                                             name=f"wgt_{fc}_{hh}") for hh in range(2)]
                            wu_h = [cwp.tile([128, KC // 2, 128], bf, tag="wucol",
                                             name=f"wut_{fc}_{hh}") for hh in range(2)]
                            for hh in range(2):
                                ksl = slice(hh * (KC // 2), (hh + 1) * (KC // 2))
                                nc.sync.dma_start(wg_h[hh][:], W[f"wg{l}"][fc][:, ksl, :])
                                nc.sync.dma_start(wu_h[hh][:], W[f"wu{l}"][fc][:, ksl, :])
                        for k in range(KC):
                            first = fc == 0 and k == 0
                            last = fc == FC - 1 and k == KC - 1
                            nc.tensor.matmul(psg[:, fc:fc + 1],
                                             wg_h[k // 8][:, k % 8, :],
                                             hn_last[:, k, :],
                                             start=first, stop=last)
                            nc.tensor.matmul(psu[:, fc:fc + 1],
                                             wu_h[k // 8][:, k % 8, :],
                                             hn_last[:, k, :],
                                             start=first, stop=last)
                    sgt = tp_.tile([128, FC], bf, tag="silut", bufs=2)
                    nc.scalar.activation(sgt[:], psg[:, 0:FC],
                                         mybir.ActivationFunctionType.Silu)
                    nc.vector.tensor_mul(m_last[:], sgt[:], psu[:, 0:FC])
                    psdn = ps.tile([128, KC], f32, tag="psum", name="psdt2")
                    for kc in range(FC):
                        if kc < len(pre["wd"]):
                            wt = pre["wd"][kc]
                        else:
                            wt = wp.tile([128, D], bf, tag="wdtail", bufs=3,
                                         name=f"wdt_{kc}")
                            nc.sync.dma_start(wt[:], W[f"wd{l}"][kc])
                        for oc in range(KC):
                            nc.tensor.matmul(
                                psdn[:, oc:oc + 1],
                                wt[:, oc * 128:(oc + 1) * 128],
                                m_last[:, kc:kc + 1],
                                start=(kc == 0 and oc == 0),
                                stop=(kc == FC - 1 and oc == KC - 1),
                            )
                    st = stp.tile([128, KC], bf, tag="stage_tail", bufs=2)
                    nc.scalar.copy(st[:], psdn[:, 0:KC])
                    nc.sync.dma_start(ar_in_s2[:, :, 0], st[:])

                # ================= layer 0 (full sequence) =================
                # per-half pipeline: k/v/q/attn/out of half 0 complete before
                # half 1 starts, so the boundary-1 collectives begin ASAP.
                # tile_wait_until values are scheduling hints only (the
                # scheduler's internal sim underestimates collective latency
                # and would otherwise head-of-line-block engine queues).
                l = 0
                nc.sync.dma_start(bq_sb[:], W[f"bq{l}"][:])
                nc.sync.dma_start(bv_sb[:], W[f"bv{l}"][:])
                nc.sync.dma_start(mstrip_sb[:], mstrip[:])
                nc.sync.dma_start(mb_sb[:], maskbias[:])
                k_proj_half(f"wk{l}", kT, 0)
                v_proj_half(l, 0)
                nc.sync.dma_start(mbt_sb[:], mbtail[:])
                nc.sync.dma_start(breg_sb[:], breg[:])
                nc.sync.dma_start(wreg_sb[:], wreg[:])
                for k4 in range(4):
                    nc.sync.dma_start(hres[:, 4 * k4:4 * (k4 + 1), :],
                                      xT[:, 4 * k4:4 * (k4 + 1), :])
                qk_attn_out_half(l, 0)
                rs_half(1, 0)
                k_proj_half(f"wk{l}", kT, 1)
                v_proj_half(l, 1)
                # boundary-1 half-0 between kv-h1 and attn-h1: its ~1us of PE
                # work slots into the PE queue while attn h1's inputs land
                with tc.tile_wait_until(wscale * 0.155):
                    boundary_stage(1, 0)
                with tc.tile_wait_until(wscale * 0.17):
                    ag_half(1, 0)
                with tc.tile_wait_until(wscale * 0.275):
                    land_hn(1, 0)
                qk_attn_out_half(l, 1)
                with tc.tile_wait_until(wscale * 0.27):
                    rs_half(1, 1)
                nc.sync.dma_start(wq1_sb[:], W["wq1"][:])
                with tc.tile_wait_until(wscale * 0.24):
                    mlp_gate_up_half(l, 0)
                with tc.tile_wait_until(wscale * 0.31):
                    boundary_stage(1, 1)
                with tc.tile_wait_until(wscale * 0.32):
                    ag_half(1, 1)
                with tc.tile_wait_until(wscale * 0.365):
                    mlp_down_half(l, 0)
                with tc.tile_wait_until(wscale * 0.405):
                    rs_half(2, 0)
                with tc.tile_wait_until(wscale * 0.39):
                    land_hn(1, 1)
                with tc.tile_wait_until(wscale * 0.40):
                    mlp_gate_up_half(l, 1, fcs=range(0, 7))
                with tc.tile_wait_until(wscale * 0.445):
                    boundary_stage(2, 0)
                with tc.tile_wait_until(wscale * 0.455):
                    ag_half(2, 0)
                with tc.tile_wait_until(wscale * 0.45):
                    mlp_gate_up_half(l, 1, fcs=range(7, FC))
                    mlp_down_half(l, 1)
                with tc.tile_wait_until(wscale * 0.525):
                    rs_half(2, 1)

                # ================= layer 1 (tail layer) =================
                l = 1
                for k in range(KC):
                    nc.sync.dma_start(aqw[:, k, :], W[f"aq{l}"][k])
                    nc.sync.dma_start(avw[:, k, :], W[f"av{l}"][k])
                nc.sync.dma_start(bq_sb[:], W[f"bq{l}"][:])
                nc.sync.dma_start(bv_sb[:], W[f"bv{l}"][:])
                with tc.tile_wait_until(wscale * 0.525):
                    land_hn(2, 0)
                with tc.tile_wait_until(wscale * 0.56):
                    lora_down_half(avw, av_sb, 0)
                    k_proj_half(f"wk{l}", kT, 0)
                    v_proj_half(l, 0)
                with tc.tile_wait_until(wscale * 0.56):
                    boundary_stage(2, 1)
                with tc.tile_wait_until(wscale * 0.575):
                    ag_half(2, 1)
                with tc.tile_wait_until(wscale * 0.645):
                    land_hn(2, 1)
                with tc.tile_wait_until(wscale * 0.71):
                    lora_down_half(avw, av_sb, 1)
                    lora_down_half(aqw, aq_sb, 1)
                    k_proj_half(f"wk{l}", kT, 1)
                    v_proj_half(l, 1)
                with tc.tile_wait_until(wscale * 0.71):
                    q_proj_tail(l)
                    attention_tail()
                    out_proj_tail(l)
                    pre = mlp_tail_preload(l)
                with tc.tile_wait_until(wscale * 0.79):
                    ar_tail(ar_in_s, ar_out_s, h_last1)
                    mlp_tail(l, pre)
                with tc.tile_wait_until(wscale * 0.79):
                    lb2 = tp_.tile([128, KC, 1], bf, tag="ar_land2")
                    ar_tail(ar_in_s2, ar_out_s2, lb2)

                    # ============= final norm + head (last token only) ========
                    hfin = tp_.tile([128, KC, 1], bf, tag="hfin")
                    nc.vector.tensor_add(hfin[:], h_last1[:], lb2[:])
                    hl = tp_.tile([128, KC, 1], bf, tag="hlast")
                    norm_tail_to(hfin, hl)
                    pso = ps.tile([128, TT], f32, tag="psum")
                    for k in range(KC):
                        nc.tensor.matmul(pso[0:OUT, 0:1], wreg_sb[:, k, :], hl[:, k, :],
                                         start=(k == 0), stop=(k == KC - 1))
                    ot = tp_.tile([OUT, 1], f32, tag="outt")
                    nc.vector.tensor_add(ot[:], pso[0:OUT, 0:1], breg_sb[:])
                    nc.sync.dma_start(out_dram[:], ot[:])

    nc.finalize()
    return nc


_CACHED = {}


def _get_program():
    if "nc" not in _CACHED:
        _CACHED["nc"] = build_program()
    return _CACHED["nc"]


def _host_prepare(inputs):
    """Fold norms/scales into weights, gather embeddings, build the 8
    per-core input maps."""
    ids = np.asarray(inputs["input_ids"]).astype(np.int64)        # [B,S]
    amask = np.asarray(inputs["attention_mask"]).astype(np.int64)  # [B,S]
    embed = np.asarray(inputs["embed"], FP32)

    inv_sqrt_hd = FP32(1.0 / np.sqrt(HD))

    # rope tables (half: both halves identical)
    inv = 1.0 / (10000.0 ** (np.arange(0, HD, 2, dtype=np.float64) / HD))
    ang = (np.arange(S, dtype=np.float64)[:, None] * inv[None, :])  # [S,64]
    cos64 = np.cos(ang).T.astype(BF16)  # [64,S]
    sin64 = np.sin(ang).T.astype(BF16)
    cosT = np.concatenate([cos64, cos64], axis=0).copy()  # [128,S]
    sinT = np.concatenate([sin64, sin64], axis=0).copy()

    # causal strip [128, 896]: strip[p,u] = 1 if (u-384) >= p else 0
    u = np.arange(896)[None, :]
    p = np.arange(128)[:, None]
    mstrip = ((u - 384) >= p).astype(BF16)

    per_core = []
    common = {}

    def fold(l):
        g1 = np.asarray(inputs["norm1"], FP32)[l][:, None]
        g2 = np.asarray(inputs["norm2"], FP32)[l][:, None]
        wq = np.asarray(inputs["Wq"], FP32)[l] * g1 * inv_sqrt_hd
        wk = np.asarray(inputs["Wk"], FP32)[l] * g1
        wv = np.asarray(inputs["Wv"], FP32)[l] * g1
        aq = np.asarray(inputs["Aq"], FP32)[l] * g1
        av = np.asarray(inputs["Av"], FP32)[l] * g1
        bq = np.asarray(inputs["Bq"], FP32)[l] * (SCALING * inv_sqrt_hd)
        bv = np.asarray(inputs["Bv"], FP32)[l] * SCALING
        wo = np.asarray(inputs["Wo"], FP32)[l]
        wg = np.asarray(inputs["Wgate"], FP32)[l] * g2
        wu = np.asarray(inputs["Wup"], FP32)[l] * g2
        wd = np.asarray(inputs["Wdown"], FP32)[l]
        return wq, wk, wv, aq, av, bq, bv, wo, wg, wu, wd

    folded = [fold(l) for l in range(L)]
    wregf = (np.asarray(inputs["Wreg"], FP32) * np.asarray(inputs["norm_f"], FP32)[:, None])
    common["wreg"] = np.ascontiguousarray(
        wregf.reshape(KC, 128, OUT).transpose(1, 0, 2)).astype(BF16)
    common["breg"] = np.asarray(inputs["breg"], FP32).reshape(OUT, 1)
    common["cosT"] = cosT
    common["sinT"] = sinT
    common["mstrip"] = mstrip

    in_maps = []
    for c in range(N_CORES):
        b = c // TP      # batch index (DP group)
        r = c % TP       # TP rank within group
        m = dict(common)
        # embedding gather, transposed: [D,S] -> [16,128,S] -> [128,16,S]
        x_ds = embed[ids[b]].T.astype(BF16).astype(FP32)  # [D, S], bf16-rounded
        xt = x_ds.reshape(KC, 128, S).transpose(1, 0, 2)
        # rank-owned residual chunks only (seq-parallel): half t, tokens
        # [t*TT + r*OW, +OW)
        own = np.concatenate(
            [xt[:, :, t * TT + r * OW:t * TT + (r + 1) * OW] for t in range(NT)],
            axis=2)
        m["xT"] = np.ascontiguousarray(own).astype(BF16)
        m["hsel"] = np.full((128, 1), 1.0 if r == TP - 1 else 0.0, dtype=BF16)
        # layer-0 rmsnorm done on host (g1 folded into weights)
        rs0 = 1.0 / np.sqrt((x_ds ** 2).mean(axis=0) + EPS)
        hn0_ds = (x_ds * rs0[None, :]).astype(BF16)  # [D,S]
        m["hn0"] = np.ascontiguousarray(
            hn0_ds.reshape(KC, 128, S).transpose(1, 0, 2))
        # layer-0 LoRA down-activations on host
        aq_f = folded[0][3]  # aq (g1-folded) [D, R]
        av_f = folded[0][4]
        hn0_f32 = hn0_ds.astype(FP32)
        m["aqx"] = np.ascontiguousarray(aq_f.T @ hn0_f32).astype(BF16)  # [R,S]
        m["avx"] = np.ascontiguousarray(av_f.T @ hn0_f32).astype(BF16)
        # attention_mask bias [128, TC]: col j, part p -> key token 128j+p
        mb = np.where(amask[b] > 0, FP32(0), FP32(-1e9)).reshape(TC, 128).T
        m["maskbias"] = np.ascontiguousarray(mb)
        m["mbtail"] = np.ascontiguousarray(np.tile(mb, (1, NH)))
        for l in range(L):
            wq, wk, wv, aq, av, bq, bv, wo, wg, wu, wd = folded[l]
            dsl = slice(r * DL, (r + 1) * DL)
            fsl = slice(r * FL, (r + 1) * FL)
            m[f"wq{l}"] = np.ascontiguousarray(
                wq[:, dsl].reshape(KC, 128, DL).transpose(1, 0, 2)).astype(BF16)
            m[f"wk{l}"] = np.ascontiguousarray(
                wk[:, dsl].reshape(KC, 128, DL).transpose(1, 0, 2)).astype(BF16)
            m[f"wv{l}"] = np.ascontiguousarray(
                wv[:, dsl].reshape(KC, 128, DL).transpose(1, 0, 2)).astype(BF16)
            m[f"aq{l}"] = np.ascontiguousarray(aq.reshape(KC, 128, R)).astype(BF16)
            m[f"av{l}"] = np.ascontiguousarray(av.reshape(KC, 128, R)).astype(BF16)
            m[f"bq{l}"] = np.ascontiguousarray(bq[:, dsl]).astype(BF16)
            m[f"bv{l}"] = np.ascontiguousarray(bv[:, dsl]).astype(BF16)
            m[f"wo{l}"] = np.ascontiguousarray(wo[dsl].reshape(NH, 128, D)).astype(BF16)
            # wg/wu: [D, FL] -> [FC, 128(d-part), KC, 128(f-col)] so the
            # per-fc SBUF load [128, KC, 128] is a contiguous DMA
            wgl = wg[:, fsl].reshape(KC, 128, FC, 128).transpose(2, 1, 0, 3)
            wul = wu[:, fsl].reshape(KC, 128, FC, 128).transpose(2, 1, 0, 3)
            m[f"wg{l}"] = np.ascontiguousarray(wgl).astype(BF16)
            m[f"wu{l}"] = np.ascontiguousarray(wul).astype(BF16)
            m[f"wd{l}"] = np.ascontiguousarray(wd[fsl].reshape(FC, 128, D)).astype(BF16)
        in_maps.append(m)
    return in_maps


def run_on_device(in_maps, trace=False):
    nc = _get_program()
    return bass_utils.run_bass_kernel_spmd(
        nc, in_maps, core_ids=list(range(N_CORES)), trace=trace,
    )


def kernel(**inputs):
    in_maps = _host_prepare(inputs)
    res = run_on_device(in_maps, trace=False)
    out = np.stack([
        res.results[0]["out"].reshape(OUT),
        res.results[TP]["out"].reshape(OUT),
    ]).astype(FP32)
    return out

